# revision 50
# baseline (speedup 1.0000x reference)
"""Lensiformer forward pass on 8 Trainium2 NeuronCores.

Strategy: data-parallel over batch (32 images -> 4 per core, params
replicated, no collectives). Per core, a single fused Bass/Tile program
runs the whole network.

  - patch embed as matmul over host-im2col'd patches (conv == matmul),
    both shifted-patch tokenizers share one matmul (f32r, as before)
  - transformer in bf16 matmuls with fp32 residual / LN / PSUM accum:
      x^T built via DMA-xbar transposes (bf16) -- no PE transposes, no
      PSUM->SBUF copies on the vector engine
      QKV for all 4 images up front; V for each image's last token
      batched into one 4-column matmul group
      attention transposed (scores^T = K^T.T Q^T per head); the three
      key-chunk score matmuls write one 3-bank PSUM tile consumed by a
      single Exp activation (scale=temp fused); diagonal/pad masking is
      a single elementwise multiply with a constant bf16 mask (split
      across DVE and GpSimd); denominator via 65th all-ones V column;
      denominators collected to an 8-partition tile, one reciprocal per
      image, broadcast via one-hot-pair matmuls
      proj/MLP bf16; gelu+bias fused in the PSUM->SBUF copy
  - final LN + head on the 4 cls tokens in fp32/f32r (unchanged)

Self-contained: includes the walrus sync-wait-limit workaround and the
axon NTFF profiling shim.
"""
import contextlib
import ctypes
import os
import sys
import types

import numpy as np

import concourse.bass as bass
import concourse.mybir as mybir
import concourse.tile as tile
from concourse.masks import make_identity
from concourse.vector_clock import ScopedClock

F32 = mybir.dt.float32
F32R = mybir.dt.float32r
BF16 = mybir.dt.bfloat16
AF = mybir.ActivationFunctionType
ALU = mybir.AluOpType

# ---------------- model geometry (hardcoded from the problem spec) ----------
B, IMG, PATCH = 32, 128, 8
D, H, L, MLP, NCLS = 512, 8, 8, 2048, 3
GRID = IMG // PATCH            # 16
P = GRID * GRID                # 256 patches / image
N = P + 1                      # 257 tokens / image
HD = D // H                    # 64
KC = 320                       # im2col contraction: 5 shifts * 8 * 8
NCORES = 8
NIMG = B // NCORES             # 4 images / core
TP = NIMG * P                  # 1024 patch tokens / core
NT = NIMG * N                  # 1028 transformer tokens / core
NTILE = 9                      # token tiles of 128
TT = NTILE * 128               # 1152 padded tokens
IMGOFF = [i * N for i in range(NIMG)]
NQ = 258                       # padded query stream per image (>=256)
NQL = 257                      # real queries written back per image

_PROGRAM_CACHE = {}

# ============================================================================
# environment fixups
# ============================================================================
_fixups_done = False


def _install_fixups():
    global _fixups_done
    if _fixups_done:
        return
    _fixups_done = True
    MAXW = 1

    def _split_waits(nc, ordered):
        for bb_name, insts in ordered.items():
            new_list = []
            for inst in insts:
                si = getattr(inst, 'sync_info', None)
                eng = getattr(inst, 'engine', None)
                if (si is not None and si.on_wait and len(si.on_wait) > MAXW
                        and eng is not None
                        and type(inst).__name__.startswith('Inst')):
                    waits = list(si.on_wait)
                    inst.sync_info = mybir.SyncInfo(
                        on_wait=waits[:MAXW], on_update=list(si.on_update or []))
                    for i in range(MAXW, len(waits), MAXW):
                        new_list.append(mybir.InstNoOp(
                            name=nc.get_next_instruction_name(),
                            engine=eng, bass_nofuse=True,
                            sync_info=mybir.SyncInfo(
                                on_wait=waits[i:i + MAXW], on_update=[])))
                new_list.append(inst)
            ordered[bb_name] = new_list

    orig_lower = tile.TileContext._lower_ordered_insts

    def patched_lower(self, ordered):
        _split_waits(self.nc, ordered)
        return orig_lower(self, ordered)

    tile.TileContext._lower_ordered_insts = patched_lower

    def patched_drain_and_barrier(self, tick_clock, wait_clock):
        drain_inst = self.nc.sync.drain()
        wait_clock.add_sem_waits(
            drain_inst.ins, ScopedClock({None: tick_clock.global_clock}))
        si = drain_inst.ins.sync_info
        if si and si.on_wait and len(si.on_wait) > MAXW:
            waits = list(si.on_wait)
            drain_inst.ins.sync_info = mybir.SyncInfo(
                on_wait=waits[:MAXW], on_update=list(si.on_update or []))
            for i in range(MAXW, len(waits), MAXW):
                extra = self.nc.sync.drain()
                extra.ins.sync_info = mybir.SyncInfo(
                    on_wait=waits[i:i + MAXW], on_update=[])
        self.nc.all_engine_barrier()
        assert self.sems is not None
        popped = self.nc._tile_sem_poison_stack.pop()
        assert popped is self._sem_poison
        self.nc.clear_and_free_semaphores(list(self.sems.allocated().values()))
        self.nc.all_engine_barrier()

    tile.TileContext._drain_and_barrier = patched_drain_and_barrier

    if 'antenv.axon_hooks' not in sys.modules:
        holder = {'h': None}
        mod = types.ModuleType('antenv.axon_hooks')
        mod.set_axon_ntff_profile_hook = lambda h: holder.__setitem__('h', h)
        mod.get_axon_ntff_profile_hook = lambda: holder['h']
        sys.modules['antenv.axon_hooks'] = mod
        try:
            lib = ctypes.CDLL('/opt/axon/libaxon_pjrt.so')
            if hasattr(lib, 'axon_start_nrt_profile'):
                lib.axon_start_nrt_profile.argtypes = [
                    ctypes.POINTER(ctypes.c_int64), ctypes.c_size_t]
                lib.axon_start_nrt_profile.restype = ctypes.c_int64
                lib.axon_stop_nrt_profile.argtypes = [ctypes.c_char_p]
                lib.axon_stop_nrt_profile.restype = ctypes.c_int64

                @contextlib.contextmanager
                def _hook(output_dir, device_ids):
                    import jax
                    jax.devices()
                    if device_ids:
                        ids = (ctypes.c_int64 * len(device_ids))(*device_ids)
                        rc = lib.axon_start_nrt_profile(ids, len(device_ids))
                    else:
                        rc = lib.axon_start_nrt_profile(None, 0)
                    if rc != 0:
                        raise RuntimeError(f'axon_start_nrt_profile rc={rc}')
                    try:
                        yield
                    finally:
                        lib.axon_stop_nrt_profile(output_dir.encode())

                mod.set_axon_ntff_profile_hook(_hook)
        except OSError:
            pass


# ============================================================================
# host-side input marshaling (pure data movement + tiny param folds)
# ============================================================================
def _im2col(image):
    """(Bc,1,IMG,IMG) -> (Bc, P, 320), col order [shift, py, px]."""
    shifts = [(0, 0), (1, 1), (-1, 1), (1, -1), (-1, -1)]
    x = image[:, 0]
    cols = []
    for (sy, sx) in shifts:
        xs = np.roll(x, (sy, sx), (1, 2))
        pt = xs.reshape(-1, GRID, PATCH, GRID, PATCH).transpose(0, 1, 3, 2, 4)
        cols.append(pt.reshape(-1, P, PATCH * PATCH))
    return np.concatenate(cols, -1)


def _rne12(a):
    """Round fp32 array to f32r (RNE at 12 low mantissa bits) - matches HW."""
    bits = np.ascontiguousarray(a, np.float32).view(np.uint32)
    half = np.uint32(1 << 11)
    mask = np.uint32((1 << 12) - 1)
    low = bits & mask
    up = (low > half) | ((low == half) & ((bits >> 12) & 1).astype(bool))
    out = ((bits & ~mask) + np.where(up, np.uint32(1 << 12), np.uint32(0)))
    return out.view(np.float32)


def _bf16(a):
    import ml_dtypes
    return np.ascontiguousarray(np.asarray(a, np.float32)).astype(
        ml_dtypes.bfloat16)


def _host_prep(inputs):
    f = lambda k: np.ascontiguousarray(np.asarray(inputs[k], np.float32))
    image = f('image')

    # conv weights -> matmul form, both tokenizers side by side
    wconv = np.concatenate(
        [f('ssw').reshape(D, KC).T, f('sow').reshape(D, KC).T], 1)  # (320,1024)
    bconv = np.concatenate([f('ssb'), f('sob')])                    # (1024,)
    gbeta = np.stack([np.concatenate([f('ssg'), f('sog')]),
                      np.concatenate([f('ssbeta'), f('sobeta')])])  # (2,1024)

    # fold LN gains/biases into the following matmuls (exact rewrite)
    ln1g, ln1b = f('ln1g'), f('ln1b')
    ln2g, ln2b = f('ln2g'), f('ln2b')
    qkvw, qkvb = f('qkvw'), f('qkvb')
    w1, b1 = f('w1'), f('b1')
    qkvw_eff = ln1g[:, :, None] * qkvw
    qkvb_eff = qkvb + np.einsum('ld,ldn->ln', ln1b, qkvw)
    w1_eff = ln2g[:, :, None] * w1
    b1_eff = b1 + np.einsum('ld,ldn->ln', ln2b, w1)
    hw_eff = f('ng')[:, None] * f('hw')

    # pos/cls in padded transformer layout
    pos = f('pos_embed')[0]          # (257, 512)
    cls_eff = f('cls_token')[0, 0] + pos[0]
    pospad = np.zeros((TT, D), np.float32)
    for i in range(NIMG):
        pospad[IMGOFF[i]] = cls_eff
        pospad[IMGOFF[i] + 1: IMGOFF[i] + N] = pos[1:]

    X = _im2col(image)               # (B, P, 320)

    # attention mask constant: maskc[p, c, q] = 0 where (key, query) masked
    maskc = np.ones((128, 3, NQ), np.float32)
    for p in range(128):
        maskc[p, 0, p] = 0.0
        maskc[p, 1, 128 + p] = 0.0
    maskc[0, 2, 256] = 0.0
    # one-hot pair lhsT for denominator broadcast:
    # ohp[k, fc*128+j] = 1 iff k == 2*fc + (j>=64)
    ohp = np.zeros((8, 512), np.float32)
    for fc in range(4):
        ohp[2 * fc, fc * 128:fc * 128 + 64] = 1.0
        ohp[2 * fc + 1, fc * 128 + 64:fc * 128 + 128] = 1.0

    rk = _rne12
    common = dict(
        maskc=_bf16(maskc), ohp=np.ascontiguousarray(ohp),
        wconv=rk(wconv), bconv=rk(bconv), gbeta=gbeta,
        fw=rk(f('fw')), fb=rk(f('fb')), pospad=pospad,
        qkvw=_bf16(qkvw_eff), qkvb=np.ascontiguousarray(qkvb_eff),
        qkvbv=_bf16(qkvb_eff[:, 2 * D:3 * D]),
        projw=_bf16(f('projw')), projb=_bf16(f('projb')), temp=f('temp'),
        w1=_bf16(w1_eff), b1=np.ascontiguousarray(b1_eff),
        w2=_bf16(f('w2')), b2=_bf16(f('b2')),
        hw=rk(np.concatenate([hw_eff, np.zeros((D, 1), np.float32)], 1)),
        hb=rk(np.concatenate([f('hb') + f('nb') @ f('hw'),
                              np.zeros(1, np.float32)])),
    )
    in_maps = []
    for c in range(NCORES):
        xt = rk(np.ascontiguousarray(
            X[c * NIMG:(c + 1) * NIMG].reshape(TP, KC).T))  # (320, 1024)
        m = dict(common)
        m['xt'] = xt
        in_maps.append(m)
    return in_maps


# ============================================================================
# device program
# ============================================================================
def _tile_segments(t):
    """Real-token segments of token-tile t: (row_in_tile, n, img, pos0)."""
    segs = []
    r0 = 128 * t
    for img in range(NIMG):
        lo = max(r0, IMGOFF[img])
        hi = min(r0 + 128, IMGOFF[img] + N, NT)
        if lo < hi:
            segs.append((lo - r0, hi - lo, img, lo - IMGOFF[img]))
    return segs


def _build_program():
    nc = bass.Bass()
    NLAYERS = int(os.environ.get('KLAYERS', str(L)))
    KDUMP = os.environ.get('KDUMP', '') == '1'

    EDT = F32R                     # embed/head matmul dtype
    TDT = BF16                     # transformer matmul dtype
    din = lambda nm, sh, dt_=F32: nc.dram_tensor(nm, sh, dt_, kind='ExternalInput')
    xt_d = din('xt', [KC, TP], EDT)
    wc_d = din('wconv', [KC, 2 * D], EDT)
    bc_d = din('bconv', [2 * D], EDT)
    gb_d = din('gbeta', [2, 2 * D])
    fw_d = din('fw', [2 * D, D], EDT)
    fb_d = din('fb', [D], EDT)
    pos_d = din('pospad', [TT, D])
    qkvw_d = din('qkvw', [L, D, 3 * D], TDT)
    qkvb_d = din('qkvb', [L, 3 * D])
    qkvbv_d = din('qkvbv', [L, D], TDT)
    projw_d = din('projw', [L, D, D], TDT)
    projb_d = din('projb', [L, D], TDT)
    temp_d = din('temp', [L, H])
    w1_d = din('w1', [L, D, MLP], TDT)
    b1_d = din('b1', [L, MLP])
    w2_d = din('w2', [L, MLP, D], TDT)
    b2_d = din('b2', [L, D], TDT)
    hw_d = din('hw', [D, 4], EDT)
    hb_d = din('hb', [4], EDT)
    maskc_d = din('maskc', [128, 3, NQ], TDT)
    ohp_d = din('ohp', [8, 512], F32R)
    out_d = nc.dram_tensor('out', [NIMG, 4], F32, kind='ExternalOutput')
    if KDUMP:
        dtok_d = nc.dram_tensor('d_tok', [NTILE, 128, D], F32,
                                kind='ExternalOutput')
        dqkta_d = nc.dram_tensor('d_qkta', [128, NIMG, 8, NQ], TDT,
                                 kind='ExternalOutput')
        dpt_d = nc.dram_tensor('d_pt', [128, 3, NQ], TDT,
                               kind='ExternalOutput')
        ddrb_d = nc.dram_tensor('d_drb', [8, NQ], F32, kind='ExternalOutput')
        dot_d = nc.dram_tensor('d_ot', [128, 4, TT], TDT,
                               kind='ExternalOutput')

    with tile.TileContext(nc) as tc, \
            nc.allow_low_precision(reason='bf16 transformer / f32r embed'):
        with contextlib.ExitStack() as ctx:
            sb = ctx.enter_context(tc.tile_pool(name='sb', bufs=1))
            ps = ctx.enter_context(tc.tile_pool(name='ps', bufs=2, space='PSUM'))

            _psn = [0]

            def pstile(tag='attq'):
                _psn[0] += 1
                bufs = 1 if tag in ('tp', 'sc') else 2
                return ps.tile([128, 512], F32, tag=tag, bufs=bufs,
                               name=f'ps{_psn[0]}')

            # ---------------- constants ----------------
            ident = sb.tile([128, 128], F32, tag='ident')
            make_identity(nc, ident)
            ident_bf = sb.tile([128, 128], BF16, tag='ident_bf')
            nc.vector.tensor_copy(out=ident_bf, in_=ident)
            onesf = sb.tile([1, 128], F32, tag='onesf')
            nc.vector.memset(onesf, 1.0)
            ones128 = sb.tile([128, 64], F32, tag='ones128')
            nc.vector.memset(ones128, 1.0)
            ones_row = sb.tile([1, 128], EDT, tag='ones_row')
            nc.vector.tensor_copy(out=ones_row, in_=onesf)
            ones_bf = sb.tile([1, 128], TDT, tag='ones_bf')
            nc.vector.tensor_copy(out=ones_bf, in_=onesf)
            eps = sb.tile([128, 1], F32, tag='eps')
            nc.vector.memset(eps, 1e-5)
            # host-built constants (partition-base rules forbid on-device)
            maskc = sb.tile([128, 3, NQ], TDT, tag='maskc')
            nc.sync.dma_start(out=maskc, in_=maskc_d[:, :, :])
            ohp = sb.tile([8, 512], F32R, tag='ohp')
            nc.sync.dma_start(out=ohp, in_=ohp_d[:, :])

            # ---------------- persistent activations ----------------
            tok = sb.tile([128, NTILE, D], F32, tag='tok')       # residual
            ot = sb.tile([128, 4, TT], TDT, tag='ot')            # attn out^T
            nc.vector.memset(ot[:, :, NT:TT], 0.0)
            # V tiles persist; their all-ones 65th columns are set once
            vimga = sb.tile([128, NIMG, 2, H * 65], TDT, tag='vimga')
            vl4 = sb.tile([1, NIMG, H * 65], TDT, tag='vl4')
            nc.vector.tensor_copy(
                out=vimga.rearrange(
                    'p i c (h e) -> p i c h e', e=65)[:, :, :, :, 64:65],
                in_=ones128[:, 0:NIMG * 2 * H])
            nc.vector.tensor_copy(
                out=vl4.rearrange(
                    'p i (h e) -> p i h e', e=65)[0:1, :, :, 64:65],
                in_=ones128[0:1, 0:NIMG * H])

            # int consts for the DVE Newton rsqrt
            magic = sb.tile([128, NTILE], mybir.dt.int32, tag='magic')
            nc.vector.memset(magic, 0x5f3759df)
            one_i = sb.tile([128, NTILE], mybir.dt.int32, tag='one_i')
            nc.vector.memset(one_i, 1)

            # rsqrt(v+eps) on the DVE (bit-trick seed + 2 Newton steps) --
            # keeps ScalarE free of Sqrt/Ln so its activation-table set
            # never thrashes between the attention Exp and the MLP Gelu.
            def _newton_rsqrt(yy, var_ap, nw):
                """yy: [128, 4, nw] scratch; writes rsqrt into yy[:, 3, :]."""
                veps = yy[:, 0, 0:nw]
                nc.vector.tensor_scalar_add(veps, var_ap, 1e-5)
                yi = yy.bitcast(mybir.dt.int32)
                nc.vector.tensor_tensor(out=yi[:, 1, 0:nw],
                                        in0=yi[:, 0, 0:nw],
                                        in1=one_i[:, 0:nw],
                                        op=ALU.logical_shift_right)
                nc.vector.tensor_tensor(out=yi[:, 1, 0:nw],
                                        in0=magic[:, 0:nw],
                                        in1=yi[:, 1, 0:nw], op=ALU.subtract)
                for it in (2, 3):
                    y = yy[:, it - 1, 0:nw]
                    nc.vector.tensor_tensor(out=yy[:, it, 0:nw], in0=y, in1=y,
                                            op=ALU.mult)
                    nc.vector.tensor_tensor(out=yy[:, it, 0:nw],
                                            in0=yy[:, it, 0:nw], in1=veps,
                                            op=ALU.mult)
                    nc.vector.tensor_scalar(out=yy[:, it, 0:nw],
                                            in0=yy[:, it, 0:nw],
                                            scalar1=-0.5, scalar2=1.5,
                                            op0=ALU.mult, op1=ALU.add)
                    nc.vector.tensor_tensor(out=yy[:, it, 0:nw], in0=y,
                                            in1=yy[:, it, 0:nw], op=ALU.mult)

            # per-tile LN helper (embed + head)
            def layer_norm_apply(src_ap, dst_ap, n_rows=128):
                nr = slice(0, n_rows)
                stats = sb.tile([128, 6], F32, tag='lnstat', bufs=4)
                mv = sb.tile([128, 2], F32, tag='lnmv', bufs=4)
                nc.vector.bn_stats(out=stats[nr], in_=src_ap)
                nc.vector.bn_aggr(out=mv[nr], in_=stats[nr])
                yy = sb.tile([128, 4, 1], F32, tag='lnyy', bufs=4)
                _newton_rsqrt(yy, mv[:, 1:2], 1)
                rstd = yy[nr, 3, 0:1]
                nmr = sb.tile([128, 1], F32, tag='lnnmr', bufs=4)
                nc.vector.scalar_tensor_tensor(
                    out=nmr[nr], in0=mv[nr, 0:1], scalar=-1.0,
                    in1=rstd, op0=ALU.mult, op1=ALU.mult)
                nc.scalar.activation(out=dst_ap, in_=src_ap, func=AF.Identity,
                                     scale=rstd, bias=nmr[nr])

            # batched LN for a whole 9-tile transformer phase: stats per
            # tile, then ONE [128, 9] rsqrt chain (the per-tile version puts
            # ~2.5us of serial [128,1] DVE ops on the critical path per tile
            # and starves the PE between phases).
            def ln_phase_stats():
                mvall = sb.tile([128, 2, NTILE], F32, tag='mvall', bufs=2)
                for t in range(NTILE):
                    stats = sb.tile([128, 6], F32, tag='lnstat', bufs=4)
                    nc.vector.bn_stats(out=stats, in_=tok[:, t, :])
                    nc.vector.bn_aggr(out=mvall[:, :, t], in_=stats)
                yy = sb.tile([128, 4, NTILE], F32, tag='lnyyb', bufs=2)
                _newton_rsqrt(yy, mvall[:, 1, :], NTILE)
                nmr = sb.tile([128, NTILE], F32, tag='lnnmrb', bufs=2)
                nc.vector.tensor_tensor(out=nmr, in0=mvall[:, 0, :],
                                        in1=yy[:, 3, :], op=ALU.mult)
                nc.vector.tensor_scalar_mul(nmr, nmr, -1.0)
                return yy, nmr

            def ln_phase_apply(yy, nmr, t, dst_ap):
                nc.scalar.activation(out=dst_ap, in_=tok[:, t, :],
                                     func=AF.Identity,
                                     scale=yy[:, 3, t:t + 1],
                                     bias=nmr[:, t:t + 1])

            # ================= patch embed (f32r, as before) =================
            with tc.tile_pool(name='emb', bufs=1) as emb:
                def psetile(tag='attq'):
                    return pstile(tag)

                xt_sb = []
                for kc, k0, kn in ((0, 0, 128), (1, 128, 128), (2, 256, 64)):
                    t_ = emb.tile([kn, TP], EDT, tag=f'xt{kc}')
                    nc.sync.dma_start(out=t_, in_=xt_d[k0:k0 + kn, :])
                    xt_sb.append(t_)
                wc_sb = []
                for kc, k0, kn in ((0, 0, 128), (1, 128, 128), (2, 256, 64)):
                    t_ = emb.tile([kn, 2 * D], EDT, tag=f'wc{kc}')
                    nc.sync.dma_start(out=t_, in_=wc_d[k0:k0 + kn, :])
                    wc_sb.append(t_)
                bc_sb = emb.tile([1, 2 * D], EDT, tag='bc')
                nc.sync.dma_start(out=bc_sb, in_=bc_d[None, :])
                gb_g = emb.tile([128, 2 * D], F32, tag='gbg')
                nc.sync.dma_start(
                    out=gb_g, in_=gb_d[0][None, :].to_broadcast([128, 2 * D]))
                gb_b = emb.tile([128, 2 * D], F32, tag='gbb')
                nc.sync.dma_start(
                    out=gb_b, in_=gb_d[1][None, :].to_broadcast([128, 2 * D]))
                fw_sb = emb.tile([128, 8, D], EDT, tag='fwsb')
                nc.sync.dma_start(
                    out=fw_sb, in_=fw_d[:, :].rearrange('(c p) n -> p c n', p=128))
                fb_sb = emb.tile([1, D], EDT, tag='fbsb')
                nc.sync.dma_start(out=fb_sb, in_=fb_d[None, :])

                fused_d = nc.dram_tensor('fusedbuf', [TP, D], F32)
                for t in range(TP // 128):      # 8 patch-layout tiles
                    combraw = emb.tile([128, 2 * D], F32, tag='combraw', bufs=2)
                    for nh in range(2):
                        cps = psetile()
                        nc.tensor.matmul(cps, ones_row[0:1, :],
                                         bc_sb[0:1, nh * D:(nh + 1) * D],
                                         start=True, stop=False,
                                         skip_group_check=True)
                        for kc in range(3):
                            nc.tensor.matmul(
                                cps, xt_sb[kc][:, t * 128:(t + 1) * 128],
                                wc_sb[kc][:, nh * D:(nh + 1) * D],
                                start=False, stop=(kc == 2),
                                skip_group_check=True)
                        nc.vector.tensor_copy(
                            out=combraw[:, nh * D:(nh + 1) * D], in_=cps)

                    comb = emb.tile([128, 2 * D], F32, tag='comb', bufs=2)
                    layer_norm_apply(combraw[:, 0:D], comb[:, 0:D])
                    layer_norm_apply(combraw[:, D:2 * D], comb[:, D:2 * D])
                    nc.vector.tensor_mul(comb, comb, gb_g)
                    nc.vector.tensor_add(comb, comb, gb_b)

                    combT = emb.tile([128, 8, 128], EDT, tag='combT', bufs=2)
                    for c in range(8):
                        tp = psetile('tp')
                        nc.tensor.transpose(tp[:, 0:128],
                                            comb[:, c * 128:(c + 1) * 128], ident)
                        nc.vector.tensor_copy(out=combT[:, c, :], in_=tp[:, 0:128])

                    gps = psetile('mlpq')
                    nc.tensor.matmul(gps, ones_row[0:1, :], fb_sb,
                                     start=True, stop=False, skip_group_check=True)
                    for c in range(8):
                        nc.tensor.matmul(gps, combT[:, c, :], fw_sb[:, c, :],
                                         start=False, stop=(c == 7),
                                         skip_group_check=True)
                    gt = emb.tile([128, D], F32, tag='gt', bufs=2)
                    nc.scalar.activation(out=gt, in_=gps, func=AF.Sigmoid)
                    diff = emb.tile([128, D], F32, tag='diff', bufs=2)
                    nc.vector.tensor_sub(diff, comb[:, 0:D], comb[:, D:2 * D])
                    nc.vector.tensor_mul(diff, diff, gt)
                    nc.vector.tensor_add(diff, diff, comb[:, D:2 * D])
                    nc.sync.dma_start(out=fused_d[t * 128:(t + 1) * 128, :],
                                      in_=diff)

                # reshuffle patch-layout fused tokens into transformer layout,
                # zero the cls rows (pos add below then yields cls_eff there)
                nc.vector.memset(tok[:, NTILE - 1, :], 0.0)
                for t in range(NTILE):
                    for (rs, nr, img, pos0) in _tile_segments(t):
                        if pos0 == 0:
                            nc.sync.dma_start(out=tok[rs:rs + 1, t, :],
                                              in_=pos_d[TT - 1:TT, :])
                            rs, nr, pos0 = rs + 1, nr - 1, 1
                        if nr <= 0:
                            continue
                        p0 = img * P + (pos0 - 1)
                        nc.sync.dma_start(out=tok[rs:rs + nr, t, :],
                                          in_=fused_d[p0:p0 + nr, :])
                    postile = emb.tile([128, D], F32, tag='pos', bufs=2)
                    nc.sync.dma_start(out=postile,
                                      in_=pos_d[t * 128:(t + 1) * 128, :])
                    nc.vector.tensor_add(tok[:, t, :], tok[:, t, :], postile)

            # ================= transformer layers (bf16) =================
            lay = ctx.enter_context(tc.tile_pool(name='lay', bufs=1))
            for l in range(NLAYERS):
                qkvw_sb = lay.tile([128, 4, 3 * D], TDT, tag='wt', bufs=3)
                nc.sync.dma_start(
                    out=qkvw_sb,
                    in_=qkvw_d[l].rearrange('(c p) n -> p c n', p=128))
                qkvb_sb = lay.tile([128, 12], F32, tag='qkvb', bufs=2)
                nc.sync.dma_start(
                    out=qkvb_sb,
                    in_=qkvb_d[l].rearrange('(c p) -> p c', p=128))
                qkvbv = lay.tile([1, D], TDT, tag='qkvbv', bufs=2)
                nc.sync.dma_start(out=qkvbv, in_=qkvbv_d[l][None, :])
                temp_sb = lay.tile([128, H], F32, tag='temp', bufs=2)
                nc.sync.dma_start(out=temp_sb,
                                  in_=temp_d[l][None, :].to_broadcast([128, H]))

                # ---- A: LN1 + transpose to feature-major ----
                xT = lay.tile([128, 4, TT], TDT, tag='xT', bufs=2)
                yyA, nmrA = ln_phase_stats()
                for t in range(NTILE):
                    xn = lay.tile([128, D], TDT, tag='xn', bufs=3)
                    ln_phase_apply(yyA, nmrA, t, xn)
                    tp4 = ps.tile([128, 4, 128], TDT, tag='tp', bufs=1,
                                  name=f'tpA{l}_{t}')
                    for c in range(4):
                        nc.tensor.matmul(tp4[:, c, :],
                                         xn[:, c * 128:(c + 1) * 128],
                                         ident_bf, is_transpose=True,
                                         skip_group_check=True)
                    nc.vector.tensor_copy(
                        out=xT[:, :, t * 128:(t + 1) * 128], in_=tp4)

                # ---- B: QKV for all images ----
                qkta = lay.tile([128, NIMG, 8, NQ], TDT, tag='qkta', bufs=1)
                for img in range(NIMG):
                    io = IMGOFF[img]
                    for fc in range(8):
                        qps = pstile()
                        for c in range(4):
                            nc.tensor.matmul(
                                qps[:, 0:NQ],
                                qkvw_sb[:, c, fc * 128:(fc + 1) * 128],
                                xT[:, c, io:io + NQ],
                                start=(c == 0), stop=(c == 3))
                        nc.scalar.activation(
                            out=qkta[:, img, fc, :], in_=qps[:, 0:NQ],
                            func=AF.Identity, bias=qkvb_sb[:, fc:fc + 1],
                            scale=1.0)
                    for c2 in range(2):
                        vp = pstile()
                        nc.tensor.matmul(vp, ones_bf[0:1, :], qkvbv,
                                         start=True, stop=False,
                                         skip_group_check=True)
                        for c in range(4):
                            nc.tensor.matmul(
                                vp, xT[:, c, io + c2 * 128:io + (c2 + 1) * 128],
                                qkvw_sb[:, c, 2 * D:3 * D],
                                start=False, stop=(c == 3),
                                skip_group_check=True)
                        nc.scalar.copy(
                            out=vimga.rearrange(
                                'p i c (h e) -> p i c h e',
                                e=65)[:, img, c2, :, 0:64],
                            in_=vp)

                # batched V for the 4 last tokens (cols 256, 513, 770, 1027);
                # engine APs can't base at partitions 1-3, so stage through
                # SBUF and let DMA do the partition scatter into vl4
                vp4 = pstile()
                nc.tensor.matmul(vp4[0:NIMG, :], ones_bf[0:1, 0:NIMG], qkvbv,
                                 start=True, stop=False, skip_group_check=True)
                for c in range(4):
                    nc.tensor.matmul(
                        vp4[0:NIMG, :], xT[:, c, P:NT:N],
                        qkvw_sb[:, c, 2 * D:3 * D],
                        start=False, stop=(c == 3), skip_group_check=True)
                vstage = lay.tile([NIMG, D], TDT, tag='vstage', bufs=2)
                nc.vector.tensor_copy(out=vstage, in_=vp4[0:NIMG, :])
                vld = nc.dram_tensor(f'vl4d{l}', [NIMG, D], TDT)
                nc.sync.dma_start(out=vld[:, :], in_=vstage)
                nc.sync.dma_start(
                    out=vl4.rearrange(
                        'p i (h e) -> p i h e', e=65)[0:1, :, :, 0:64],
                    in_=vld.rearrange('i (h e) -> i h e', e=64)[None])

                # ---- C: attention (transposed softmax) ----
                for img in range(NIMG):
                    io = IMGOFF[img]
                    drs = lay.tile([1, 8, NQ], F32, tag='drs', bufs=2)
                    for h in range(H):
                        hr = (h % 2) * 64
                        qfc = h // 2
                        kfc = 4 + h // 2
                        sc = ps.tile([128, 3, 512], F32, tag='sc', bufs=1,
                                     name=f'sc{l}_{img}_{h}')
                        for c in range(3):
                            cm = (128, 128, 1)[c]
                            nc.tensor.matmul(
                                sc[0:cm, c, 0:NQ],
                                qkta[hr:hr + 64, img, kfc,
                                     c * 128:c * 128 + cm],
                                qkta[hr:hr + 64, img, qfc, :],
                                start=True, stop=True, skip_group_check=True)
                        pt = lay.tile([128, 3, NQ], TDT, tag='pt', bufs=3)
                        nc.scalar.activation(out=pt, in_=sc[:, :, 0:NQ],
                                             func=AF.Exp,
                                             scale=temp_sb[:, h:h + 1])
                        meng = nc.gpsimd if (h % 2 == 0) else nc.vector
                        meng.tensor_mul(pt, pt, maskc)
                        if KDUMP and l == 0 and img == 0 and h == 0:
                            nc.sync.dma_start(out=dpt_d[:, :, :], in_=pt)
                        pvp = pstile()
                        for c in range(3):
                            cm = (128, 128, 1)[c]
                            lhs = (vimga[0:128, img, c, h * 65:h * 65 + 65]
                                   if c < 2
                                   else vl4[0:1, img, h * 65:h * 65 + 65])
                            nc.tensor.matmul(
                                pvp[0:65, 0:NQ], lhs, pt[0:cm, c, :],
                                start=(c == 0), stop=(c == 2),
                                skip_group_check=True)
                        nc.vector.tensor_copy(
                            out=ot[hr:hr + 64, h // 2, io:io + NQL],
                            in_=pvp[0:64, 0:NQL])
                        nc.vector.tensor_copy(out=drs[0:1, h, :],
                                              in_=pvp[64:65, 0:NQ])
                    # partition-scatter the 8 denominator rows (via DRAM --
                    # engine APs cannot base at partitions 1-7), then one
                    # 8-partition reciprocal
                    drd = nc.dram_tensor(f'drd{l}_{img}', [8, NQ], F32)
                    nc.sync.dma_start(out=drd[None, :, :], in_=drs[0:1, :, :])
                    drb = lay.tile([8, NQ], F32, tag='drb', bufs=2)
                    nc.sync.dma_start(out=drb, in_=drd[:, :])
                    rr = lay.tile([8, NQ], F32R, tag='rr', bufs=2)
                    nc.vector.reciprocal(out=rr, in_=drb)
                    if KDUMP and l == 0 and img == 0:
                        nc.sync.dma_start(out=ddrb_d[:, :], in_=drb)
                    for fc in range(4):
                        rtp = pstile()
                        nc.tensor.matmul(rtp[:, 0:NQ],
                                         ohp[:, fc * 128:(fc + 1) * 128], rr,
                                         start=True, stop=True,
                                         skip_group_check=True)
                        nc.vector.tensor_mul(ot[:, fc, io:io + NQL],
                                             ot[:, fc, io:io + NQL],
                                             rtp[:, 0:NQL])

                if KDUMP and l == 0:
                    nc.sync.dma_start(out=dqkta_d[:, :, :, :], in_=qkta)
                    nc.sync.dma_start(out=dot_d[:, :, :], in_=ot)

                # ---- D: proj + residual ----
                projw_sb = lay.tile([128, 4, D], TDT, tag='wt', bufs=3)
                nc.sync.dma_start(
                    out=projw_sb,
                    in_=projw_d[l].rearrange('(c p) n -> p c n', p=128))
                projb_sb = lay.tile([1, D], TDT, tag='projb', bufs=2)
                nc.sync.dma_start(out=projb_sb, in_=projb_d[l][None, :])
                for t in range(NTILE):
                    pp = pstile('mlpq')
                    nc.tensor.matmul(pp, ones_bf[0:1, :], projb_sb,
                                     start=True, stop=False,
                                     skip_group_check=True)
                    for c in range(4):
                        nc.tensor.matmul(pp, ot[:, c, t * 128:(t + 1) * 128],
                                         projw_sb[:, c, :],
                                         start=False, stop=(c == 3),
                                         skip_group_check=True)
                    nc.vector.tensor_add(tok[:, t, :], tok[:, t, :], pp)

                # ---- E: LN2 + transpose ----
                xT = lay.tile([128, 4, TT], TDT, tag='xT', bufs=2)
                yyE, nmrE = ln_phase_stats()
                for t in range(NTILE):
                    xn = lay.tile([128, D], TDT, tag='xn', bufs=3)
                    ln_phase_apply(yyE, nmrE, t, xn)
                    tp4 = ps.tile([128, 4, 128], TDT, tag='tp', bufs=1,
                                  name=f'tpE{l}_{t}')
                    for c in range(4):
                        nc.tensor.matmul(tp4[:, c, :],
                                         xn[:, c * 128:(c + 1) * 128],
                                         ident_bf, is_transpose=True,
                                         skip_group_check=True)
                    nc.vector.tensor_copy(
                        out=xT[:, :, t * 128:(t + 1) * 128], in_=tp4)

                # ---- F/G: MLP ----
                w1_sb = lay.tile([128, 4, MLP], TDT, tag='wt', bufs=3)
                nc.sync.dma_start(
                    out=w1_sb, in_=w1_d[l].rearrange('(c p) n -> p c n', p=128))
                b1_sb = lay.tile([128, 16], F32, tag='b1', bufs=2)
                nc.sync.dma_start(
                    out=b1_sb, in_=b1_d[l].rearrange('(c p) -> p c', p=128))
                w2_sb = lay.tile([128, 16, D], TDT, tag='wt', bufs=3)
                nc.sync.dma_start(
                    out=w2_sb, in_=w2_d[l].rearrange('(c p) n -> p c n', p=128))
                b2_sb = lay.tile([1, D], TDT, tag='b2', bufs=2)
                nc.sync.dma_start(out=b2_sb, in_=b2_d[l][None, :])

                for g in range(3):
                    g0 = g * 384
                    gw = 384
                    hT = lay.tile([128, 16, 384], TDT, tag='hT', bufs=1)
                    for hc in range(16):
                        hp = pstile('mlpq')
                        for c in range(4):
                            nc.tensor.matmul(
                                hp[:, 0:gw],
                                w1_sb[:, c, hc * 128:(hc + 1) * 128],
                                xT[:, c, g0:g0 + gw],
                                start=(c == 0), stop=(c == 3))
                        nc.scalar.activation(
                            out=hT[:, hc, 0:gw], in_=hp[:, 0:gw], func=AF.Gelu,
                            bias=b1_sb[:, hc:hc + 1], scale=1.0)
                    for tr in range(gw // 128):
                        t = (g0 + tr * 128) // 128
                        mp = pstile('mlpq')
                        nc.tensor.matmul(mp, ones_bf[0:1, :], b2_sb,
                                         start=True, stop=False,
                                         skip_group_check=True)
                        for c in range(16):
                            nc.tensor.matmul(
                                mp, hT[:, c, tr * 128:(tr + 1) * 128],
                                w2_sb[:, c, :],
                                start=False, stop=(c == 15),
                                skip_group_check=True)
                        nc.vector.tensor_add(tok[:, t, :], tok[:, t, :], mp)

            if KDUMP:
                for t in range(NTILE):
                    nc.sync.dma_start(out=dtok_d[t, :, :], in_=tok[:, t, :])

            # ================= head =================
            hw_sb = lay.tile([128, 4, 4], EDT, tag='hwsb')
            nc.sync.dma_start(out=hw_sb,
                              in_=hw_d[:, :].rearrange('(c p) n -> p c n', p=128))
            hb_sb = lay.tile([1, 4], EDT, tag='hbsb')
            nc.sync.dma_start(out=hb_sb, in_=hb_d[None, :])

            cls_sb = lay.tile([NIMG, D], F32, tag='cls')
            for img in range(NIMG):
                r = IMGOFF[img]
                nc.sync.dma_start(out=cls_sb[img:img + 1, :],
                                  in_=tok[r % 128:r % 128 + 1, r // 128, :])
            clsn = lay.tile([NIMG, D], F32, tag='clsn')
            layer_norm_apply(cls_sb[0:NIMG, :], clsn[0:NIMG, :], n_rows=NIMG)
            clsT = lay.tile([128, 4, NIMG], EDT, tag='clsT')
            for c in range(4):
                tp = pstile('mlpq')
                nc.tensor.transpose(tp[0:128, 0:NIMG],
                                    clsn[0:NIMG, c * 128:(c + 1) * 128],
                                    ident[0:NIMG, 0:NIMG])
                nc.vector.tensor_copy(out=clsT[:, c, :], in_=tp[0:128, 0:NIMG])
            op = pstile('mlpq')
            nc.tensor.matmul(op[0:NIMG, 0:4], ones_row[0:1, 0:NIMG], hb_sb,
                             start=True, stop=False, skip_group_check=True)
            for c in range(4):
                nc.tensor.matmul(op[0:NIMG, 0:4], clsT[:, c, :],
                                 hw_sb[:, c, :],
                                 start=False, stop=(c == 3),
                                 skip_group_check=True)
            osb = lay.tile([NIMG, 4], F32, tag='osb')
            nc.vector.tensor_copy(out=osb[0:NIMG, :], in_=op[0:NIMG, 0:4])
            nc.sync.dma_start(out=out_d[:, :], in_=osb[0:NIMG, :])

    return nc


# ============================================================================
# entry point
# ============================================================================
def kernel(**inputs) -> np.ndarray:
    _install_fixups()
    from concourse.bass_utils import run_bass_kernel_spmd

    key = ('nc', os.environ.get('KLAYERS', ''), os.environ.get('KDUMP', ''))
    if key not in _PROGRAM_CACHE:
        _PROGRAM_CACHE[key] = _build_program()
    nc = _PROGRAM_CACHE[key]
    _PROGRAM_CACHE['nc'] = nc

    in_maps = _host_prep(inputs)
    res = run_bass_kernel_spmd(nc, in_maps, core_ids=list(range(NCORES)))
    out = np.concatenate([np.asarray(res.results[i]['out'])
                          for i in range(NCORES)], 0)
    return out[:, :NCLS].astype(np.float32)


# revision 53
# speedup vs baseline: 1.0929x; 1.0929x over previous
"""Lensiformer forward pass on 8 Trainium2 NeuronCores.

Strategy: data-parallel over batch (32 images -> 4 per core, params
replicated, no collectives). Per core, a single fused Bass/Tile program
runs the whole network.

  - patch embed as matmul over host-im2col'd patches (conv == matmul),
    both shifted-patch tokenizers share one matmul (f32r, as before)
  - transformer in bf16 matmuls with fp32 residual / LN / PSUM accum:
      x^T built via DMA-xbar transposes (bf16) -- no PE transposes, no
      PSUM->SBUF copies on the vector engine
      QKV for all 4 images up front; V for each image's last token
      batched into one 4-column matmul group
      attention transposed (scores^T = K^T.T Q^T per head); the three
      key-chunk score matmuls write one 3-bank PSUM tile consumed by a
      single Exp activation (scale=temp fused); diagonal/pad masking is
      a single elementwise multiply with a constant bf16 mask (split
      across DVE and GpSimd); denominator via 65th all-ones V column;
      denominators collected to an 8-partition tile, one reciprocal per
      image, broadcast via one-hot-pair matmuls
      proj/MLP bf16; gelu+bias fused in the PSUM->SBUF copy
  - final LN + head on the 4 cls tokens in fp32/f32r (unchanged)

Self-contained: includes the walrus sync-wait-limit workaround and the
axon NTFF profiling shim.
"""
import contextlib
import ctypes
import os
import sys
import types

import numpy as np

import concourse.bass as bass
import concourse.mybir as mybir
import concourse.tile as tile
from concourse.masks import make_identity
from concourse.vector_clock import ScopedClock

F32 = mybir.dt.float32
F32R = mybir.dt.float32r
BF16 = mybir.dt.bfloat16
AF = mybir.ActivationFunctionType
ALU = mybir.AluOpType

# ---------------- model geometry (hardcoded from the problem spec) ----------
B, IMG, PATCH = 32, 128, 8
D, H, L, MLP, NCLS = 512, 8, 8, 2048, 3
GRID = IMG // PATCH            # 16
P = GRID * GRID                # 256 patches / image
N = P + 1                      # 257 tokens / image
HD = D // H                    # 64
KC = 320                       # im2col contraction: 5 shifts * 8 * 8
NCORES = 8
NIMG = B // NCORES             # 4 images / core
TP = NIMG * P                  # 1024 patch tokens / core
NT = NIMG * N                  # 1028 transformer tokens / core
NTILE = 9                      # token tiles of 128
TT = NTILE * 128               # 1152 padded tokens
IMGOFF = [i * N for i in range(NIMG)]
NQ = 258                       # padded query stream per image (>=256)
NQL = 257                      # real queries written back per image

_PROGRAM_CACHE = {}

# ============================================================================
# environment fixups
# ============================================================================
_fixups_done = False


def _install_fixups():
    global _fixups_done
    if _fixups_done:
        return
    _fixups_done = True
    MAXW = 1

    def _split_waits(nc, ordered):
        for bb_name, insts in ordered.items():
            new_list = []
            for inst in insts:
                si = getattr(inst, 'sync_info', None)
                eng = getattr(inst, 'engine', None)
                if (si is not None and si.on_wait and len(si.on_wait) > MAXW
                        and eng is not None
                        and type(inst).__name__.startswith('Inst')):
                    waits = list(si.on_wait)
                    inst.sync_info = mybir.SyncInfo(
                        on_wait=waits[:MAXW], on_update=list(si.on_update or []))
                    for i in range(MAXW, len(waits), MAXW):
                        new_list.append(mybir.InstNoOp(
                            name=nc.get_next_instruction_name(),
                            engine=eng, bass_nofuse=True,
                            sync_info=mybir.SyncInfo(
                                on_wait=waits[i:i + MAXW], on_update=[])))
                new_list.append(inst)
            ordered[bb_name] = new_list

    orig_lower = tile.TileContext._lower_ordered_insts

    def patched_lower(self, ordered):
        _split_waits(self.nc, ordered)
        return orig_lower(self, ordered)

    tile.TileContext._lower_ordered_insts = patched_lower

    def patched_drain_and_barrier(self, tick_clock, wait_clock):
        drain_inst = self.nc.sync.drain()
        wait_clock.add_sem_waits(
            drain_inst.ins, ScopedClock({None: tick_clock.global_clock}))
        si = drain_inst.ins.sync_info
        if si and si.on_wait and len(si.on_wait) > MAXW:
            waits = list(si.on_wait)
            drain_inst.ins.sync_info = mybir.SyncInfo(
                on_wait=waits[:MAXW], on_update=list(si.on_update or []))
            for i in range(MAXW, len(waits), MAXW):
                extra = self.nc.sync.drain()
                extra.ins.sync_info = mybir.SyncInfo(
                    on_wait=waits[i:i + MAXW], on_update=[])
        self.nc.all_engine_barrier()
        assert self.sems is not None
        popped = self.nc._tile_sem_poison_stack.pop()
        assert popped is self._sem_poison
        self.nc.clear_and_free_semaphores(list(self.sems.allocated().values()))
        self.nc.all_engine_barrier()

    tile.TileContext._drain_and_barrier = patched_drain_and_barrier

    if 'antenv.axon_hooks' not in sys.modules:
        holder = {'h': None}
        mod = types.ModuleType('antenv.axon_hooks')
        mod.set_axon_ntff_profile_hook = lambda h: holder.__setitem__('h', h)
        mod.get_axon_ntff_profile_hook = lambda: holder['h']
        sys.modules['antenv.axon_hooks'] = mod
        try:
            lib = ctypes.CDLL('/opt/axon/libaxon_pjrt.so')
            if hasattr(lib, 'axon_start_nrt_profile'):
                lib.axon_start_nrt_profile.argtypes = [
                    ctypes.POINTER(ctypes.c_int64), ctypes.c_size_t]
                lib.axon_start_nrt_profile.restype = ctypes.c_int64
                lib.axon_stop_nrt_profile.argtypes = [ctypes.c_char_p]
                lib.axon_stop_nrt_profile.restype = ctypes.c_int64

                @contextlib.contextmanager
                def _hook(output_dir, device_ids):
                    import jax
                    jax.devices()
                    if device_ids:
                        ids = (ctypes.c_int64 * len(device_ids))(*device_ids)
                        rc = lib.axon_start_nrt_profile(ids, len(device_ids))
                    else:
                        rc = lib.axon_start_nrt_profile(None, 0)
                    if rc != 0:
                        raise RuntimeError(f'axon_start_nrt_profile rc={rc}')
                    try:
                        yield
                    finally:
                        lib.axon_stop_nrt_profile(output_dir.encode())

                mod.set_axon_ntff_profile_hook(_hook)
        except OSError:
            pass


# ============================================================================
# host-side input marshaling (pure data movement + tiny param folds)
# ============================================================================
def _im2col(image):
    """(Bc,1,IMG,IMG) -> (Bc, P, 320), col order [shift, py, px]."""
    shifts = [(0, 0), (1, 1), (-1, 1), (1, -1), (-1, -1)]
    x = image[:, 0]
    cols = []
    for (sy, sx) in shifts:
        xs = np.roll(x, (sy, sx), (1, 2))
        pt = xs.reshape(-1, GRID, PATCH, GRID, PATCH).transpose(0, 1, 3, 2, 4)
        cols.append(pt.reshape(-1, P, PATCH * PATCH))
    return np.concatenate(cols, -1)


def _rne12(a):
    """Round fp32 array to f32r (RNE at 12 low mantissa bits) - matches HW."""
    bits = np.ascontiguousarray(a, np.float32).view(np.uint32)
    half = np.uint32(1 << 11)
    mask = np.uint32((1 << 12) - 1)
    low = bits & mask
    up = (low > half) | ((low == half) & ((bits >> 12) & 1).astype(bool))
    out = ((bits & ~mask) + np.where(up, np.uint32(1 << 12), np.uint32(0)))
    return out.view(np.float32)


def _bf16(a):
    import ml_dtypes
    return np.ascontiguousarray(np.asarray(a, np.float32)).astype(
        ml_dtypes.bfloat16)


def _host_prep(inputs):
    f = lambda k: np.ascontiguousarray(np.asarray(inputs[k], np.float32))
    image = f('image')

    # conv weights -> matmul form, both tokenizers side by side
    wconv = np.concatenate(
        [f('ssw').reshape(D, KC).T, f('sow').reshape(D, KC).T], 1)  # (320,1024)
    bconv = np.concatenate([f('ssb'), f('sob')])                    # (1024,)
    gbeta = np.stack([np.concatenate([f('ssg'), f('sog')]),
                      np.concatenate([f('ssbeta'), f('sobeta')])])  # (2,1024)

    # fold LN gains/biases into the following matmuls (exact rewrite)
    ln1g, ln1b = f('ln1g'), f('ln1b')
    ln2g, ln2b = f('ln2g'), f('ln2b')
    qkvw, qkvb = f('qkvw'), f('qkvb')
    w1, b1 = f('w1'), f('b1')
    qkvw_eff = ln1g[:, :, None] * qkvw
    qkvb_eff = qkvb + np.einsum('ld,ldn->ln', ln1b, qkvw)
    w1_eff = ln2g[:, :, None] * w1
    b1_eff = b1 + np.einsum('ld,ldn->ln', ln2b, w1)
    hw_eff = f('ng')[:, None] * f('hw')

    # pos/cls in padded transformer layout
    pos = f('pos_embed')[0]          # (257, 512)
    cls_eff = f('cls_token')[0, 0] + pos[0]
    pospad = np.zeros((TT, D), np.float32)
    for i in range(NIMG):
        pospad[IMGOFF[i]] = cls_eff
        pospad[IMGOFF[i] + 1: IMGOFF[i] + N] = pos[1:]

    X = _im2col(image)               # (B, P, 320)

    # attention mask constant: maskc[p, c, q] = 0 where (key, query) masked
    maskc = np.ones((128, 3, NQ), np.float32)
    for p in range(128):
        maskc[p, 0, p] = 0.0
        maskc[p, 1, 128 + p] = 0.0
    maskc[0, 2, 256] = 0.0
    # one-hot pair lhsT for denominator broadcast:
    # ohp[k, fc*128+j] = 1 iff k == 2*fc + (j>=64)
    ohp = np.zeros((8, 512), np.float32)
    for fc in range(4):
        ohp[2 * fc, fc * 128:fc * 128 + 64] = 1.0
        ohp[2 * fc + 1, fc * 128 + 64:fc * 128 + 128] = 1.0

    rk = _rne12
    common = dict(
        maskc=_bf16(maskc), ohp=np.ascontiguousarray(ohp),
        wconv=rk(wconv), bconv=rk(bconv), gbeta=gbeta,
        fw=rk(f('fw')), fb=rk(f('fb')), pospad=pospad,
        qkvw=_bf16(qkvw_eff), qkvb=np.ascontiguousarray(qkvb_eff),
        qkvbv=_bf16(qkvb_eff[:, 2 * D:3 * D]),
        projw=_bf16(f('projw')), projb=_bf16(f('projb')), temp=f('temp'),
        w1=_bf16(w1_eff), b1=np.ascontiguousarray(b1_eff),
        w2=_bf16(f('w2')), b2=_bf16(f('b2')),
        hw=rk(np.concatenate([hw_eff, np.zeros((D, 1), np.float32)], 1)),
        hb=rk(np.concatenate([f('hb') + f('nb') @ f('hw'),
                              np.zeros(1, np.float32)])),
    )
    in_maps = []
    for c in range(NCORES):
        xt = rk(np.ascontiguousarray(
            X[c * NIMG:(c + 1) * NIMG].reshape(TP, KC).T))  # (320, 1024)
        m = dict(common)
        m['xt'] = xt
        in_maps.append(m)
    return in_maps


# ============================================================================
# device program
# ============================================================================
def _tile_segments(t):
    """Real-token segments of token-tile t: (row_in_tile, n, img, pos0)."""
    segs = []
    r0 = 128 * t
    for img in range(NIMG):
        lo = max(r0, IMGOFF[img])
        hi = min(r0 + 128, IMGOFF[img] + N, NT)
        if lo < hi:
            segs.append((lo - r0, hi - lo, img, lo - IMGOFF[img]))
    return segs


def _build_program():
    nc = bass.Bass()
    NLAYERS = int(os.environ.get('KLAYERS', str(L)))
    KDUMP = os.environ.get('KDUMP', '') == '1'

    EDT = F32R                     # embed/head matmul dtype
    TDT = BF16                     # transformer matmul dtype
    din = lambda nm, sh, dt_=F32: nc.dram_tensor(nm, sh, dt_, kind='ExternalInput')
    xt_d = din('xt', [KC, TP], EDT)
    wc_d = din('wconv', [KC, 2 * D], EDT)
    bc_d = din('bconv', [2 * D], EDT)
    gb_d = din('gbeta', [2, 2 * D])
    fw_d = din('fw', [2 * D, D], EDT)
    fb_d = din('fb', [D], EDT)
    pos_d = din('pospad', [TT, D])
    qkvw_d = din('qkvw', [L, D, 3 * D], TDT)
    qkvb_d = din('qkvb', [L, 3 * D])
    qkvbv_d = din('qkvbv', [L, D], TDT)
    projw_d = din('projw', [L, D, D], TDT)
    projb_d = din('projb', [L, D], TDT)
    temp_d = din('temp', [L, H])
    w1_d = din('w1', [L, D, MLP], TDT)
    b1_d = din('b1', [L, MLP])
    w2_d = din('w2', [L, MLP, D], TDT)
    b2_d = din('b2', [L, D], TDT)
    hw_d = din('hw', [D, 4], EDT)
    hb_d = din('hb', [4], EDT)
    maskc_d = din('maskc', [128, 3, NQ], TDT)
    ohp_d = din('ohp', [8, 512], F32R)
    out_d = nc.dram_tensor('out', [NIMG, 4], F32, kind='ExternalOutput')
    if KDUMP:
        dtok_d = nc.dram_tensor('d_tok', [NTILE, 128, D], F32,
                                kind='ExternalOutput')
        dqkta_d = nc.dram_tensor('d_qkta', [128, NIMG, 8, NQ], TDT,
                                 kind='ExternalOutput')
        dpt_d = nc.dram_tensor('d_pt', [128, 3, NQ], TDT,
                               kind='ExternalOutput')
        ddrb_d = nc.dram_tensor('d_drb', [8, NQ], F32, kind='ExternalOutput')
        dot_d = nc.dram_tensor('d_ot', [128, 4, TT], TDT,
                               kind='ExternalOutput')

    with tile.TileContext(nc) as tc, \
            nc.allow_low_precision(reason='bf16 transformer / f32r embed'):
        with contextlib.ExitStack() as ctx:
            sb = ctx.enter_context(tc.tile_pool(name='sb', bufs=1))
            ps = ctx.enter_context(tc.tile_pool(name='ps', bufs=2, space='PSUM'))

            _psn = [0]

            def pstile(tag='attq'):
                _psn[0] += 1
                bufs = 1 if tag in ('tp', 'sc') else 2
                return ps.tile([128, 512], F32, tag=tag, bufs=bufs,
                               name=f'ps{_psn[0]}')

            # ---------------- constants ----------------
            ident = sb.tile([128, 128], F32, tag='ident')
            make_identity(nc, ident)
            ident_bf = sb.tile([128, 128], BF16, tag='ident_bf')
            nc.vector.tensor_copy(out=ident_bf, in_=ident)
            onesf = sb.tile([1, 128], F32, tag='onesf')
            nc.vector.memset(onesf, 1.0)
            ones128 = sb.tile([128, 64], F32, tag='ones128')
            nc.vector.memset(ones128, 1.0)
            ones_row = sb.tile([1, 128], EDT, tag='ones_row')
            nc.vector.tensor_copy(out=ones_row, in_=onesf)
            ones_bf = sb.tile([1, 128], TDT, tag='ones_bf')
            nc.vector.tensor_copy(out=ones_bf, in_=onesf)
            eps = sb.tile([128, 1], F32, tag='eps')
            nc.vector.memset(eps, 1e-5)
            # host-built constants (partition-base rules forbid on-device)
            maskc = sb.tile([128, 3, NQ], TDT, tag='maskc')
            nc.sync.dma_start(out=maskc, in_=maskc_d[:, :, :])
            ohp = sb.tile([8, 512], F32R, tag='ohp')
            nc.sync.dma_start(out=ohp, in_=ohp_d[:, :])

            # ---------------- persistent activations ----------------
            tok = sb.tile([128, NTILE, D], F32, tag='tok')       # residual
            ot = sb.tile([128, 4, TT], TDT, tag='ot')            # attn out^T
            nc.vector.memset(ot[:, :, NT:TT], 0.0)
            # V tiles persist; their all-ones 65th columns are set once
            vimga = sb.tile([128, NIMG, 2, H * 65], TDT, tag='vimga')
            vl4 = sb.tile([1, NIMG, H * 65], TDT, tag='vl4')
            nc.vector.tensor_copy(
                out=vimga.rearrange(
                    'p i c (h e) -> p i c h e', e=65)[:, :, :, :, 64:65],
                in_=ones128[:, 0:NIMG * 2 * H])
            nc.vector.tensor_copy(
                out=vl4.rearrange(
                    'p i (h e) -> p i h e', e=65)[0:1, :, :, 64:65],
                in_=ones128[0:1, 0:NIMG * H])

            # int consts for the DVE Newton rsqrt
            magic = sb.tile([128, NTILE], mybir.dt.int32, tag='magic')
            nc.vector.memset(magic, 0x5f3759df)
            one_i = sb.tile([128, NTILE], mybir.dt.int32, tag='one_i')
            nc.vector.memset(one_i, 1)

            # rsqrt(v+eps) on the DVE (bit-trick seed + 2 Newton steps) --
            # keeps ScalarE free of Sqrt/Ln so its activation-table set
            # never thrashes between the attention Exp and the MLP Gelu.
            def _newton_rsqrt(yy, var_ap, nw):
                """yy: [128, 4, nw] scratch; writes rsqrt into yy[:, 3, :]."""
                veps = yy[:, 0, 0:nw]
                nc.vector.tensor_scalar_add(veps, var_ap, 1e-5)
                yi = yy.bitcast(mybir.dt.int32)
                nc.vector.tensor_tensor(out=yi[:, 1, 0:nw],
                                        in0=yi[:, 0, 0:nw],
                                        in1=one_i[:, 0:nw],
                                        op=ALU.logical_shift_right)
                nc.vector.tensor_tensor(out=yi[:, 1, 0:nw],
                                        in0=magic[:, 0:nw],
                                        in1=yi[:, 1, 0:nw], op=ALU.subtract)
                for it in (2, 3):
                    y = yy[:, it - 1, 0:nw]
                    nc.vector.tensor_tensor(out=yy[:, it, 0:nw], in0=y, in1=y,
                                            op=ALU.mult)
                    nc.vector.tensor_tensor(out=yy[:, it, 0:nw],
                                            in0=yy[:, it, 0:nw], in1=veps,
                                            op=ALU.mult)
                    nc.vector.tensor_scalar(out=yy[:, it, 0:nw],
                                            in0=yy[:, it, 0:nw],
                                            scalar1=-0.5, scalar2=1.5,
                                            op0=ALU.mult, op1=ALU.add)
                    nc.vector.tensor_tensor(out=yy[:, it, 0:nw], in0=y,
                                            in1=yy[:, it, 0:nw], op=ALU.mult)

            # per-tile LN helper (embed + head)
            def layer_norm_apply(src_ap, dst_ap, n_rows=128):
                nr = slice(0, n_rows)
                stats = sb.tile([128, 6], F32, tag='lnstat', bufs=4)
                mv = sb.tile([128, 2], F32, tag='lnmv', bufs=4)
                nc.vector.bn_stats(out=stats[nr], in_=src_ap)
                nc.vector.bn_aggr(out=mv[nr], in_=stats[nr])
                yy = sb.tile([128, 4, 1], F32, tag='lnyy', bufs=4)
                _newton_rsqrt(yy, mv[:, 1:2], 1)
                rstd = yy[nr, 3, 0:1]
                nmr = sb.tile([128, 1], F32, tag='lnnmr', bufs=4)
                nc.vector.scalar_tensor_tensor(
                    out=nmr[nr], in0=mv[nr, 0:1], scalar=-1.0,
                    in1=rstd, op0=ALU.mult, op1=ALU.mult)
                nc.scalar.activation(out=dst_ap, in_=src_ap, func=AF.Identity,
                                     scale=rstd, bias=nmr[nr])

            # batched LN for a whole 9-tile transformer phase: stats per
            # tile, then ONE [128, 9] rsqrt chain (the per-tile version puts
            # ~2.5us of serial [128,1] DVE ops on the critical path per tile
            # and starves the PE between phases).
            def ln_phase_stats():
                mvall = sb.tile([128, 2, NTILE], F32, tag='mvall', bufs=2)
                for t in range(NTILE):
                    stats = sb.tile([128, 6], F32, tag='lnstat', bufs=4)
                    nc.vector.bn_stats(out=stats, in_=tok[:, t, :])
                    nc.vector.bn_aggr(out=mvall[:, :, t], in_=stats)
                yy = sb.tile([128, 4, NTILE], F32, tag='lnyyb', bufs=2)
                _newton_rsqrt(yy, mvall[:, 1, :], NTILE)
                nmr = sb.tile([128, NTILE], F32, tag='lnnmrb', bufs=2)
                nc.vector.tensor_tensor(out=nmr, in0=mvall[:, 0, :],
                                        in1=yy[:, 3, :], op=ALU.mult)
                nc.vector.tensor_scalar_mul(nmr, nmr, -1.0)
                return yy, nmr

            def ln_phase_apply(yy, nmr, t, dst_ap):
                nc.scalar.activation(out=dst_ap, in_=tok[:, t, :],
                                     func=AF.Identity,
                                     scale=yy[:, 3, t:t + 1],
                                     bias=nmr[:, t:t + 1])

            # ================= patch embed (f32r, as before) =================
            with tc.tile_pool(name='emb', bufs=1) as emb:
                def psetile(tag='attq'):
                    return pstile(tag)

                xt_sb = []
                for kc, k0, kn in ((0, 0, 128), (1, 128, 128), (2, 256, 64)):
                    t_ = emb.tile([kn, TP], EDT, tag=f'xt{kc}')
                    nc.sync.dma_start(out=t_, in_=xt_d[k0:k0 + kn, :])
                    xt_sb.append(t_)
                wc_sb = []
                for kc, k0, kn in ((0, 0, 128), (1, 128, 128), (2, 256, 64)):
                    t_ = emb.tile([kn, 2 * D], EDT, tag=f'wc{kc}')
                    nc.sync.dma_start(out=t_, in_=wc_d[k0:k0 + kn, :])
                    wc_sb.append(t_)
                bc_sb = emb.tile([1, 2 * D], EDT, tag='bc')
                nc.sync.dma_start(out=bc_sb, in_=bc_d[None, :])
                gb_g = emb.tile([128, 2 * D], F32, tag='gbg')
                nc.sync.dma_start(
                    out=gb_g, in_=gb_d[0][None, :].to_broadcast([128, 2 * D]))
                gb_b = emb.tile([128, 2 * D], F32, tag='gbb')
                nc.sync.dma_start(
                    out=gb_b, in_=gb_d[1][None, :].to_broadcast([128, 2 * D]))
                fw_sb = emb.tile([128, 8, D], EDT, tag='fwsb')
                nc.sync.dma_start(
                    out=fw_sb, in_=fw_d[:, :].rearrange('(c p) n -> p c n', p=128))
                fb_sb = emb.tile([1, D], EDT, tag='fbsb')
                nc.sync.dma_start(out=fb_sb, in_=fb_d[None, :])

                fused_d = nc.dram_tensor('fusedbuf', [TP, D], F32)
                for t in range(TP // 128):      # 8 patch-layout tiles
                    combraw = emb.tile([128, 2 * D], F32, tag='combraw', bufs=2)
                    for nh in range(2):
                        cps = psetile()
                        nc.tensor.matmul(cps, ones_row[0:1, :],
                                         bc_sb[0:1, nh * D:(nh + 1) * D],
                                         start=True, stop=False,
                                         skip_group_check=True)
                        for kc in range(3):
                            nc.tensor.matmul(
                                cps, xt_sb[kc][:, t * 128:(t + 1) * 128],
                                wc_sb[kc][:, nh * D:(nh + 1) * D],
                                start=False, stop=(kc == 2),
                                skip_group_check=True)
                        nc.vector.tensor_copy(
                            out=combraw[:, nh * D:(nh + 1) * D], in_=cps)

                    comb = emb.tile([128, 2 * D], F32, tag='comb', bufs=2)
                    layer_norm_apply(combraw[:, 0:D], comb[:, 0:D])
                    layer_norm_apply(combraw[:, D:2 * D], comb[:, D:2 * D])
                    nc.vector.tensor_mul(comb, comb, gb_g)
                    nc.vector.tensor_add(comb, comb, gb_b)

                    combT = emb.tile([128, 8, 128], EDT, tag='combT', bufs=2)
                    for c in range(8):
                        tp = psetile('tp')
                        nc.tensor.transpose(tp[:, 0:128],
                                            comb[:, c * 128:(c + 1) * 128], ident)
                        nc.vector.tensor_copy(out=combT[:, c, :], in_=tp[:, 0:128])

                    gps = psetile('mlpq')
                    nc.tensor.matmul(gps, ones_row[0:1, :], fb_sb,
                                     start=True, stop=False, skip_group_check=True)
                    for c in range(8):
                        nc.tensor.matmul(gps, combT[:, c, :], fw_sb[:, c, :],
                                         start=False, stop=(c == 7),
                                         skip_group_check=True)
                    gt = emb.tile([128, D], F32, tag='gt', bufs=2)
                    nc.scalar.activation(out=gt, in_=gps, func=AF.Sigmoid)
                    diff = emb.tile([128, D], F32, tag='diff', bufs=2)
                    nc.vector.tensor_sub(diff, comb[:, 0:D], comb[:, D:2 * D])
                    nc.vector.tensor_mul(diff, diff, gt)
                    nc.vector.tensor_add(diff, diff, comb[:, D:2 * D])
                    nc.sync.dma_start(out=fused_d[t * 128:(t + 1) * 128, :],
                                      in_=diff)

                # reshuffle patch-layout fused tokens into transformer layout,
                # zero the cls rows (pos add below then yields cls_eff there)
                nc.vector.memset(tok[:, NTILE - 1, :], 0.0)
                for t in range(NTILE):
                    for (rs, nr, img, pos0) in _tile_segments(t):
                        if pos0 == 0:
                            nc.sync.dma_start(out=tok[rs:rs + 1, t, :],
                                              in_=pos_d[TT - 1:TT, :])
                            rs, nr, pos0 = rs + 1, nr - 1, 1
                        if nr <= 0:
                            continue
                        p0 = img * P + (pos0 - 1)
                        nc.sync.dma_start(out=tok[rs:rs + nr, t, :],
                                          in_=fused_d[p0:p0 + nr, :])
                    postile = emb.tile([128, D], F32, tag='pos', bufs=2)
                    nc.sync.dma_start(out=postile,
                                      in_=pos_d[t * 128:(t + 1) * 128, :])
                    nc.vector.tensor_add(tok[:, t, :], tok[:, t, :], postile)

            # ================= transformer layers (bf16) =================
            lay = ctx.enter_context(tc.tile_pool(name='lay', bufs=1))
            for l in range(NLAYERS):
                qkvw_sb = lay.tile([128, 4, 3 * D], TDT, tag='wt', bufs=3)
                nc.sync.dma_start(
                    out=qkvw_sb,
                    in_=qkvw_d[l].rearrange('(c p) n -> p c n', p=128))
                qkvb_sb = lay.tile([128, 12], F32, tag='qkvb', bufs=2)
                nc.sync.dma_start(
                    out=qkvb_sb,
                    in_=qkvb_d[l].rearrange('(c p) -> p c', p=128))
                qkvbv = lay.tile([1, D], TDT, tag='qkvbv', bufs=2)
                nc.sync.dma_start(out=qkvbv, in_=qkvbv_d[l][None, :])
                temp_sb = lay.tile([128, H], F32, tag='temp', bufs=2)
                nc.sync.dma_start(out=temp_sb,
                                  in_=temp_d[l][None, :].to_broadcast([128, H]))

                # ---- A: LN1 + transpose to feature-major ----
                xT = lay.tile([128, 4, TT], TDT, tag='xT', bufs=2)
                yyA, nmrA = ln_phase_stats()
                for t in range(NTILE):
                    xn = lay.tile([128, D], TDT, tag='xn', bufs=3)
                    ln_phase_apply(yyA, nmrA, t, xn)
                    tp4 = ps.tile([128, 4, 128], TDT, tag='tp', bufs=1,
                                  name=f'tpA{l}_{t}')
                    for c in range(4):
                        nc.tensor.matmul(tp4[:, c, :],
                                         xn[:, c * 128:(c + 1) * 128],
                                         ident_bf, is_transpose=True,
                                         skip_group_check=True)
                    nc.vector.tensor_copy(
                        out=xT[:, :, t * 128:(t + 1) * 128], in_=tp4)

                # ---- B: QKV for all images ----
                qkta = lay.tile([128, NIMG, 8, NQ], TDT, tag='qkta', bufs=1)
                for img in range(NIMG):
                    if img == 1:
                        # batched V for the 4 last tokens (cols 256..1027).
                        # Issued after img0's QKV: its DRAM partition-scatter
                        # roundtrip (engine APs cannot base at partitions
                        # 1-3) has ~8us latency that must hide under the
                        # remaining QKV matmuls, not block the attention
                        # FIFO at PV(img0, h0, chunk2).
                        vp4 = pstile()
                        nc.tensor.matmul(vp4[0:NIMG, :], ones_bf[0:1, 0:NIMG],
                                         qkvbv, start=True, stop=False,
                                         skip_group_check=True)
                        for c in range(4):
                            nc.tensor.matmul(
                                vp4[0:NIMG, :], xT[:, c, P:NT:N],
                                qkvw_sb[:, c, 2 * D:3 * D],
                                start=False, stop=(c == 3),
                                skip_group_check=True)
                        vstage = lay.tile([NIMG, D], TDT, tag='vstage',
                                          bufs=2)
                        nc.vector.tensor_copy(out=vstage, in_=vp4[0:NIMG, :])
                        vld = nc.dram_tensor(f'vl4d{l}', [NIMG, D], TDT)
                        nc.sync.dma_start(out=vld[:, :], in_=vstage)
                        nc.sync.dma_start(
                            out=vl4.rearrange(
                                'p i (h e) -> p i h e', e=65)[0:1, :, :, 0:64],
                            in_=vld.rearrange('i (h e) -> i h e', e=64)[None])
                    io = IMGOFF[img]
                    for fc in range(8):
                        qps = pstile()
                        for c in range(4):
                            nc.tensor.matmul(
                                qps[:, 0:NQ],
                                qkvw_sb[:, c, fc * 128:(fc + 1) * 128],
                                xT[:, c, io:io + NQ],
                                start=(c == 0), stop=(c == 3))
                        nc.scalar.activation(
                            out=qkta[:, img, fc, :], in_=qps[:, 0:NQ],
                            func=AF.Identity, bias=qkvb_sb[:, fc:fc + 1],
                            scale=1.0)
                    for c2 in range(2):
                        vp = pstile()
                        nc.tensor.matmul(vp, ones_bf[0:1, :], qkvbv,
                                         start=True, stop=False,
                                         skip_group_check=True)
                        for c in range(4):
                            nc.tensor.matmul(
                                vp, xT[:, c, io + c2 * 128:io + (c2 + 1) * 128],
                                qkvw_sb[:, c, 2 * D:3 * D],
                                start=False, stop=(c == 3),
                                skip_group_check=True)
                        nc.scalar.copy(
                            out=vimga.rearrange(
                                'p i c (h e) -> p i c h e',
                                e=65)[:, img, c2, :, 0:64],
                            in_=vp)


                # ---- C: attention (transposed softmax) ----
                drb_l = []
                for img in range(NIMG):
                    io = IMGOFF[img]
                    drs = lay.tile([1, 8, NQ], F32, tag='drs', bufs=4)
                    for h in range(H):
                        hr = (h % 2) * 64
                        qfc = h // 2
                        kfc = 4 + h // 2
                        sc = ps.tile([128, 3, 512], F32, tag='sc', bufs=1,
                                     name=f'sc{l}_{img}_{h}')
                        for c in range(3):
                            cm = (128, 128, 1)[c]
                            nc.tensor.matmul(
                                sc[0:cm, c, 0:NQ],
                                qkta[hr:hr + 64, img, kfc,
                                     c * 128:c * 128 + cm],
                                qkta[hr:hr + 64, img, qfc, :],
                                start=True, stop=True, skip_group_check=True)
                        pt = lay.tile([128, 3, NQ], TDT, tag='pt', bufs=3)
                        nc.scalar.activation(out=pt, in_=sc[:, :, 0:NQ],
                                             func=AF.Exp,
                                             scale=temp_sb[:, h:h + 1])
                        meng = nc.gpsimd if (h % 2 == 0) else nc.vector
                        meng.tensor_mul(pt, pt, maskc)
                        if KDUMP and l == 0 and img == 0 and h == 0:
                            nc.sync.dma_start(out=dpt_d[:, :, :], in_=pt)
                        pvp = pstile()
                        for c in range(3):
                            cm = (128, 128, 1)[c]
                            lhs = (vimga[0:128, img, c, h * 65:h * 65 + 65]
                                   if c < 2
                                   else vl4[0:1, img, h * 65:h * 65 + 65])
                            nc.tensor.matmul(
                                pvp[0:65, 0:NQ], lhs, pt[0:cm, c, :],
                                start=(c == 0), stop=(c == 2),
                                skip_group_check=True)
                        nc.vector.tensor_copy(
                            out=ot[hr:hr + 64, h // 2, io:io + NQL],
                            in_=pvp[0:64, 0:NQL])
                        nc.vector.tensor_copy(out=drs[0:1, h, :],
                                              in_=pvp[64:65, 0:NQ])
                    # partition-scatter the 8 denominator rows (via DRAM --
                    # engine APs cannot base at partitions 1-7); the
                    # reciprocal + broadcast happen in a second pass so this
                    # ~8us DMA chain never blocks the PE FIFO ahead of the
                    # next image's score matmuls
                    drd = nc.dram_tensor(f'drd{l}_{img}', [8, NQ], F32)
                    nc.sync.dma_start(out=drd[None, :, :], in_=drs[0:1, :, :])
                    drb = lay.tile([8, NQ], F32, tag='drb', bufs=4)
                    nc.sync.dma_start(out=drb, in_=drd[:, :])
                    drb_l.append(drb)

                for img in range(NIMG):
                    io = IMGOFF[img]
                    rr = lay.tile([8, NQ], F32R, tag='rr', bufs=2)
                    nc.vector.reciprocal(out=rr, in_=drb_l[img])
                    if KDUMP and l == 0 and img == 0:
                        nc.sync.dma_start(out=ddrb_d[:, :], in_=drb_l[img])
                    for fc in range(4):
                        rtp = pstile()
                        nc.tensor.matmul(rtp[:, 0:NQ],
                                         ohp[:, fc * 128:(fc + 1) * 128], rr,
                                         start=True, stop=True,
                                         skip_group_check=True)
                        nc.vector.tensor_mul(ot[:, fc, io:io + NQL],
                                             ot[:, fc, io:io + NQL],
                                             rtp[:, 0:NQL])

                if KDUMP and l == 0:
                    nc.sync.dma_start(out=dqkta_d[:, :, :, :], in_=qkta)
                    nc.sync.dma_start(out=dot_d[:, :, :], in_=ot)

                # ---- D: proj + residual ----
                projw_sb = lay.tile([128, 4, D], TDT, tag='wt', bufs=3)
                nc.sync.dma_start(
                    out=projw_sb,
                    in_=projw_d[l].rearrange('(c p) n -> p c n', p=128))
                projb_sb = lay.tile([1, D], TDT, tag='projb', bufs=2)
                nc.sync.dma_start(out=projb_sb, in_=projb_d[l][None, :])
                for t in range(NTILE):
                    pp = pstile('mlpq')
                    nc.tensor.matmul(pp, ones_bf[0:1, :], projb_sb,
                                     start=True, stop=False,
                                     skip_group_check=True)
                    for c in range(4):
                        nc.tensor.matmul(pp, ot[:, c, t * 128:(t + 1) * 128],
                                         projw_sb[:, c, :],
                                         start=False, stop=(c == 3),
                                         skip_group_check=True)
                    nc.vector.tensor_add(tok[:, t, :], tok[:, t, :], pp)

                # ---- E: LN2 + transpose ----
                xT = lay.tile([128, 4, TT], TDT, tag='xT', bufs=2)
                yyE, nmrE = ln_phase_stats()
                for t in range(NTILE):
                    xn = lay.tile([128, D], TDT, tag='xn', bufs=3)
                    ln_phase_apply(yyE, nmrE, t, xn)
                    tp4 = ps.tile([128, 4, 128], TDT, tag='tp', bufs=1,
                                  name=f'tpE{l}_{t}')
                    for c in range(4):
                        nc.tensor.matmul(tp4[:, c, :],
                                         xn[:, c * 128:(c + 1) * 128],
                                         ident_bf, is_transpose=True,
                                         skip_group_check=True)
                    nc.vector.tensor_copy(
                        out=xT[:, :, t * 128:(t + 1) * 128], in_=tp4)

                # ---- F/G: MLP ----
                w1_sb = lay.tile([128, 4, MLP], TDT, tag='wt', bufs=3)
                nc.sync.dma_start(
                    out=w1_sb, in_=w1_d[l].rearrange('(c p) n -> p c n', p=128))
                b1_sb = lay.tile([128, 16], F32, tag='b1', bufs=2)
                nc.sync.dma_start(
                    out=b1_sb, in_=b1_d[l].rearrange('(c p) -> p c', p=128))
                w2_sb = lay.tile([128, 16, D], TDT, tag='wt', bufs=3)
                nc.sync.dma_start(
                    out=w2_sb, in_=w2_d[l].rearrange('(c p) n -> p c n', p=128))
                b2_sb = lay.tile([1, D], TDT, tag='b2', bufs=2)
                nc.sync.dma_start(out=b2_sb, in_=b2_d[l][None, :])

                for g in range(3):
                    g0 = g * 384
                    gw = 384
                    hT = lay.tile([128, 16, 384], TDT, tag='hT', bufs=1)
                    for hc in range(16):
                        hp = pstile('mlpq')
                        for c in range(4):
                            nc.tensor.matmul(
                                hp[:, 0:gw],
                                w1_sb[:, c, hc * 128:(hc + 1) * 128],
                                xT[:, c, g0:g0 + gw],
                                start=(c == 0), stop=(c == 3))
                        nc.scalar.activation(
                            out=hT[:, hc, 0:gw], in_=hp[:, 0:gw], func=AF.Gelu,
                            bias=b1_sb[:, hc:hc + 1], scale=1.0)
                    for tr in range(gw // 128):
                        t = (g0 + tr * 128) // 128
                        mp = pstile('mlpq')
                        nc.tensor.matmul(mp, ones_bf[0:1, :], b2_sb,
                                         start=True, stop=False,
                                         skip_group_check=True)
                        for c in range(16):
                            nc.tensor.matmul(
                                mp, hT[:, c, tr * 128:(tr + 1) * 128],
                                w2_sb[:, c, :],
                                start=False, stop=(c == 15),
                                skip_group_check=True)
                        nc.vector.tensor_add(tok[:, t, :], tok[:, t, :], mp)

            if KDUMP:
                for t in range(NTILE):
                    nc.sync.dma_start(out=dtok_d[t, :, :], in_=tok[:, t, :])

            # ================= head =================
            hw_sb = lay.tile([128, 4, 4], EDT, tag='hwsb')
            nc.sync.dma_start(out=hw_sb,
                              in_=hw_d[:, :].rearrange('(c p) n -> p c n', p=128))
            hb_sb = lay.tile([1, 4], EDT, tag='hbsb')
            nc.sync.dma_start(out=hb_sb, in_=hb_d[None, :])

            cls_sb = lay.tile([NIMG, D], F32, tag='cls')
            for img in range(NIMG):
                r = IMGOFF[img]
                nc.sync.dma_start(out=cls_sb[img:img + 1, :],
                                  in_=tok[r % 128:r % 128 + 1, r // 128, :])
            clsn = lay.tile([NIMG, D], F32, tag='clsn')
            layer_norm_apply(cls_sb[0:NIMG, :], clsn[0:NIMG, :], n_rows=NIMG)
            clsT = lay.tile([128, 4, NIMG], EDT, tag='clsT')
            for c in range(4):
                tp = pstile('mlpq')
                nc.tensor.transpose(tp[0:128, 0:NIMG],
                                    clsn[0:NIMG, c * 128:(c + 1) * 128],
                                    ident[0:NIMG, 0:NIMG])
                nc.vector.tensor_copy(out=clsT[:, c, :], in_=tp[0:128, 0:NIMG])
            op = pstile('mlpq')
            nc.tensor.matmul(op[0:NIMG, 0:4], ones_row[0:1, 0:NIMG], hb_sb,
                             start=True, stop=False, skip_group_check=True)
            for c in range(4):
                nc.tensor.matmul(op[0:NIMG, 0:4], clsT[:, c, :],
                                 hw_sb[:, c, :],
                                 start=False, stop=(c == 3),
                                 skip_group_check=True)
            osb = lay.tile([NIMG, 4], F32, tag='osb')
            nc.vector.tensor_copy(out=osb[0:NIMG, :], in_=op[0:NIMG, 0:4])
            nc.sync.dma_start(out=out_d[:, :], in_=osb[0:NIMG, :])

    return nc


# ============================================================================
# entry point
# ============================================================================
def kernel(**inputs) -> np.ndarray:
    _install_fixups()
    from concourse.bass_utils import run_bass_kernel_spmd

    key = ('nc', os.environ.get('KLAYERS', ''), os.environ.get('KDUMP', ''))
    if key not in _PROGRAM_CACHE:
        _PROGRAM_CACHE[key] = _build_program()
    nc = _PROGRAM_CACHE[key]
    _PROGRAM_CACHE['nc'] = nc

    in_maps = _host_prep(inputs)
    res = run_bass_kernel_spmd(nc, in_maps, core_ids=list(range(NCORES)))
    out = np.concatenate([np.asarray(res.results[i]['out'])
                          for i in range(NCORES)], 0)
    return out[:, :NCLS].astype(np.float32)


# revision 55
# speedup vs baseline: 1.1523x; 1.0543x over previous
"""Lensiformer forward pass on 8 Trainium2 NeuronCores.

Strategy: data-parallel over batch (32 images -> 4 per core, params
replicated, no collectives). Per core, a single fused Bass/Tile program
runs the whole network.

  - patch embed as matmul over host-im2col'd patches (conv == matmul),
    both shifted-patch tokenizers share one matmul (f32r, as before)
  - transformer in bf16 matmuls with fp32 residual / LN / PSUM accum:
      x^T built via DMA-xbar transposes (bf16) -- no PE transposes, no
      PSUM->SBUF copies on the vector engine
      QKV for all 4 images up front; V for each image's last token
      batched into one 4-column matmul group
      attention transposed (scores^T = K^T.T Q^T per head); the three
      key-chunk score matmuls write one 3-bank PSUM tile consumed by a
      single Exp activation (scale=temp fused); diagonal/pad masking is
      a single elementwise multiply with a constant bf16 mask (split
      across DVE and GpSimd); denominator via 65th all-ones V column;
      denominators collected to an 8-partition tile, one reciprocal per
      image, broadcast via one-hot-pair matmuls
      proj/MLP bf16; gelu+bias fused in the PSUM->SBUF copy
  - final LN + head on the 4 cls tokens in fp32/f32r (unchanged)

Self-contained: includes the walrus sync-wait-limit workaround and the
axon NTFF profiling shim.
"""
import contextlib
import ctypes
import os
import sys
import types

import numpy as np

import concourse.bass as bass
import concourse.mybir as mybir
import concourse.tile as tile
from concourse.masks import make_identity
from concourse.vector_clock import ScopedClock

F32 = mybir.dt.float32
F32R = mybir.dt.float32r
BF16 = mybir.dt.bfloat16
AF = mybir.ActivationFunctionType
ALU = mybir.AluOpType

# ---------------- model geometry (hardcoded from the problem spec) ----------
B, IMG, PATCH = 32, 128, 8
D, H, L, MLP, NCLS = 512, 8, 8, 2048, 3
GRID = IMG // PATCH            # 16
P = GRID * GRID                # 256 patches / image
N = P + 1                      # 257 tokens / image
HD = D // H                    # 64
KC = 320                       # im2col contraction: 5 shifts * 8 * 8
NCORES = 8
NIMG = B // NCORES             # 4 images / core
TP = NIMG * P                  # 1024 patch tokens / core
NT = NIMG * N                  # 1028 transformer tokens / core
NTILE = 9                      # token tiles of 128
TT = NTILE * 128               # 1152 padded tokens
IMGOFF = [i * N for i in range(NIMG)]
NQ = 258                       # padded query stream per image (>=256)
NQL = 257                      # real queries written back per image

_PROGRAM_CACHE = {}

# ============================================================================
# environment fixups
# ============================================================================
_fixups_done = False


def _install_fixups():
    global _fixups_done
    if _fixups_done:
        return
    _fixups_done = True
    MAXW = 1

    def _split_waits(nc, ordered):
        for bb_name, insts in ordered.items():
            new_list = []
            for inst in insts:
                si = getattr(inst, 'sync_info', None)
                eng = getattr(inst, 'engine', None)
                if (si is not None and si.on_wait and len(si.on_wait) > MAXW
                        and eng is not None
                        and type(inst).__name__.startswith('Inst')):
                    waits = list(si.on_wait)
                    inst.sync_info = mybir.SyncInfo(
                        on_wait=waits[:MAXW], on_update=list(si.on_update or []))
                    for i in range(MAXW, len(waits), MAXW):
                        new_list.append(mybir.InstNoOp(
                            name=nc.get_next_instruction_name(),
                            engine=eng, bass_nofuse=True,
                            sync_info=mybir.SyncInfo(
                                on_wait=waits[i:i + MAXW], on_update=[])))
                new_list.append(inst)
            ordered[bb_name] = new_list

    orig_lower = tile.TileContext._lower_ordered_insts

    def patched_lower(self, ordered):
        _split_waits(self.nc, ordered)
        return orig_lower(self, ordered)

    tile.TileContext._lower_ordered_insts = patched_lower

    def patched_drain_and_barrier(self, tick_clock, wait_clock):
        drain_inst = self.nc.sync.drain()
        wait_clock.add_sem_waits(
            drain_inst.ins, ScopedClock({None: tick_clock.global_clock}))
        si = drain_inst.ins.sync_info
        if si and si.on_wait and len(si.on_wait) > MAXW:
            waits = list(si.on_wait)
            drain_inst.ins.sync_info = mybir.SyncInfo(
                on_wait=waits[:MAXW], on_update=list(si.on_update or []))
            for i in range(MAXW, len(waits), MAXW):
                extra = self.nc.sync.drain()
                extra.ins.sync_info = mybir.SyncInfo(
                    on_wait=waits[i:i + MAXW], on_update=[])
        self.nc.all_engine_barrier()
        assert self.sems is not None
        popped = self.nc._tile_sem_poison_stack.pop()
        assert popped is self._sem_poison
        self.nc.clear_and_free_semaphores(list(self.sems.allocated().values()))
        self.nc.all_engine_barrier()

    tile.TileContext._drain_and_barrier = patched_drain_and_barrier

    if 'antenv.axon_hooks' not in sys.modules:
        holder = {'h': None}
        mod = types.ModuleType('antenv.axon_hooks')
        mod.set_axon_ntff_profile_hook = lambda h: holder.__setitem__('h', h)
        mod.get_axon_ntff_profile_hook = lambda: holder['h']
        sys.modules['antenv.axon_hooks'] = mod
        try:
            lib = ctypes.CDLL('/opt/axon/libaxon_pjrt.so')
            if hasattr(lib, 'axon_start_nrt_profile'):
                lib.axon_start_nrt_profile.argtypes = [
                    ctypes.POINTER(ctypes.c_int64), ctypes.c_size_t]
                lib.axon_start_nrt_profile.restype = ctypes.c_int64
                lib.axon_stop_nrt_profile.argtypes = [ctypes.c_char_p]
                lib.axon_stop_nrt_profile.restype = ctypes.c_int64

                @contextlib.contextmanager
                def _hook(output_dir, device_ids):
                    import jax
                    jax.devices()
                    if device_ids:
                        ids = (ctypes.c_int64 * len(device_ids))(*device_ids)
                        rc = lib.axon_start_nrt_profile(ids, len(device_ids))
                    else:
                        rc = lib.axon_start_nrt_profile(None, 0)
                    if rc != 0:
                        raise RuntimeError(f'axon_start_nrt_profile rc={rc}')
                    try:
                        yield
                    finally:
                        lib.axon_stop_nrt_profile(output_dir.encode())

                mod.set_axon_ntff_profile_hook(_hook)
        except OSError:
            pass


# ============================================================================
# host-side input marshaling (pure data movement + tiny param folds)
# ============================================================================
def _im2col(image):
    """(Bc,1,IMG,IMG) -> (Bc, P, 320), col order [shift, py, px]."""
    shifts = [(0, 0), (1, 1), (-1, 1), (1, -1), (-1, -1)]
    x = image[:, 0]
    cols = []
    for (sy, sx) in shifts:
        xs = np.roll(x, (sy, sx), (1, 2))
        pt = xs.reshape(-1, GRID, PATCH, GRID, PATCH).transpose(0, 1, 3, 2, 4)
        cols.append(pt.reshape(-1, P, PATCH * PATCH))
    return np.concatenate(cols, -1)


def _rne12(a):
    """Round fp32 array to f32r (RNE at 12 low mantissa bits) - matches HW."""
    bits = np.ascontiguousarray(a, np.float32).view(np.uint32)
    half = np.uint32(1 << 11)
    mask = np.uint32((1 << 12) - 1)
    low = bits & mask
    up = (low > half) | ((low == half) & ((bits >> 12) & 1).astype(bool))
    out = ((bits & ~mask) + np.where(up, np.uint32(1 << 12), np.uint32(0)))
    return out.view(np.float32)


def _bf16(a):
    import ml_dtypes
    return np.ascontiguousarray(np.asarray(a, np.float32)).astype(
        ml_dtypes.bfloat16)


def _host_prep(inputs):
    f = lambda k: np.ascontiguousarray(np.asarray(inputs[k], np.float32))
    image = f('image')

    # conv weights -> matmul form, both tokenizers side by side
    wconv = np.concatenate(
        [f('ssw').reshape(D, KC).T, f('sow').reshape(D, KC).T], 1)  # (320,1024)
    bconv = np.concatenate([f('ssb'), f('sob')])                    # (1024,)
    gbeta = np.stack([np.concatenate([f('ssg'), f('sog')]),
                      np.concatenate([f('ssbeta'), f('sobeta')])])  # (2,1024)

    # fold LN gains/biases into the following matmuls (exact rewrite)
    ln1g, ln1b = f('ln1g'), f('ln1b')
    ln2g, ln2b = f('ln2g'), f('ln2b')
    qkvw, qkvb = f('qkvw'), f('qkvb')
    w1, b1 = f('w1'), f('b1')
    qkvw_eff = ln1g[:, :, None] * qkvw
    qkvb_eff = qkvb + np.einsum('ld,ldn->ln', ln1b, qkvw)
    w1_eff = ln2g[:, :, None] * w1
    b1_eff = b1 + np.einsum('ld,ldn->ln', ln2b, w1)
    hw_eff = f('ng')[:, None] * f('hw')

    # pos/cls in padded transformer layout
    pos = f('pos_embed')[0]          # (257, 512)
    cls_eff = f('cls_token')[0, 0] + pos[0]
    pospad = np.zeros((TT, D), np.float32)
    for i in range(NIMG):
        pospad[IMGOFF[i]] = cls_eff
        pospad[IMGOFF[i] + 1: IMGOFF[i] + N] = pos[1:]

    X = _im2col(image)               # (B, P, 320)

    # attention mask constant: maskc[p, c, q] = 0 where (key, query) masked
    maskc = np.ones((128, 3, NQ), np.float32)
    for p in range(128):
        maskc[p, 0, p] = 0.0
        maskc[p, 1, 128 + p] = 0.0
    maskc[0, 2, 256] = 0.0
    # one-hot pair lhsT for denominator broadcast:
    # ohp[k, fc*128+j] = 1 iff k == 2*fc + (j>=64)
    ohp = np.zeros((8, 512), np.float32)
    for fc in range(4):
        ohp[2 * fc, fc * 128:fc * 128 + 64] = 1.0
        ohp[2 * fc + 1, fc * 128 + 64:fc * 128 + 128] = 1.0

    rk = _rne12
    common = dict(
        maskc=_bf16(maskc), ohp=np.ascontiguousarray(ohp),
        wconv=rk(wconv), bconv=rk(bconv), gbeta=gbeta,
        fw=rk(f('fw')), fb=rk(f('fb')), pospad=pospad,
        qkvw=_bf16(qkvw_eff), qkvb=np.ascontiguousarray(qkvb_eff),
        qkvbv=_bf16(qkvb_eff[:, 2 * D:3 * D]),
        projw=_bf16(f('projw')), projb=_bf16(f('projb')), temp=f('temp'),
        w1=_bf16(w1_eff), b1=np.ascontiguousarray(b1_eff),
        w2=_bf16(f('w2')), b2=_bf16(f('b2')),
        hw=rk(np.concatenate([hw_eff, np.zeros((D, 1), np.float32)], 1)),
        hb=rk(np.concatenate([f('hb') + f('nb') @ f('hw'),
                              np.zeros(1, np.float32)])),
    )
    in_maps = []
    for c in range(NCORES):
        xt = rk(np.ascontiguousarray(
            X[c * NIMG:(c + 1) * NIMG].reshape(TP, KC).T))  # (320, 1024)
        m = dict(common)
        m['xt'] = xt
        in_maps.append(m)
    return in_maps


# ============================================================================
# device program
# ============================================================================
def _tile_segments(t):
    """Real-token segments of token-tile t: (row_in_tile, n, img, pos0)."""
    segs = []
    r0 = 128 * t
    for img in range(NIMG):
        lo = max(r0, IMGOFF[img])
        hi = min(r0 + 128, IMGOFF[img] + N, NT)
        if lo < hi:
            segs.append((lo - r0, hi - lo, img, lo - IMGOFF[img]))
    return segs


def _build_program():
    nc = bass.Bass()
    NLAYERS = int(os.environ.get('KLAYERS', str(L)))
    KDUMP = os.environ.get('KDUMP', '') == '1'

    EDT = F32R                     # embed/head matmul dtype
    TDT = BF16                     # transformer matmul dtype
    din = lambda nm, sh, dt_=F32: nc.dram_tensor(nm, sh, dt_, kind='ExternalInput')
    xt_d = din('xt', [KC, TP], EDT)
    wc_d = din('wconv', [KC, 2 * D], EDT)
    bc_d = din('bconv', [2 * D], EDT)
    gb_d = din('gbeta', [2, 2 * D])
    fw_d = din('fw', [2 * D, D], EDT)
    fb_d = din('fb', [D], EDT)
    pos_d = din('pospad', [TT, D])
    qkvw_d = din('qkvw', [L, D, 3 * D], TDT)
    qkvb_d = din('qkvb', [L, 3 * D])
    qkvbv_d = din('qkvbv', [L, D], TDT)
    projw_d = din('projw', [L, D, D], TDT)
    projb_d = din('projb', [L, D], TDT)
    temp_d = din('temp', [L, H])
    w1_d = din('w1', [L, D, MLP], TDT)
    b1_d = din('b1', [L, MLP])
    w2_d = din('w2', [L, MLP, D], TDT)
    b2_d = din('b2', [L, D], TDT)
    hw_d = din('hw', [D, 4], EDT)
    hb_d = din('hb', [4], EDT)
    maskc_d = din('maskc', [128, 3, NQ], TDT)
    ohp_d = din('ohp', [8, 512], F32R)
    out_d = nc.dram_tensor('out', [NIMG, 4], F32, kind='ExternalOutput')
    if KDUMP:
        dtok_d = nc.dram_tensor('d_tok', [NTILE, 128, D], F32,
                                kind='ExternalOutput')
        dqkta_d = nc.dram_tensor('d_qkta', [128, NIMG, 8, NQ], TDT,
                                 kind='ExternalOutput')
        dpt_d = nc.dram_tensor('d_pt', [128, 3, NQ], TDT,
                               kind='ExternalOutput')
        ddrb_d = nc.dram_tensor('d_drb', [8, NQ], F32, kind='ExternalOutput')
        dot_d = nc.dram_tensor('d_ot', [128, 4, TT], TDT,
                               kind='ExternalOutput')

    with tile.TileContext(nc) as tc, \
            nc.allow_low_precision(reason='bf16 transformer / f32r embed'):
        with contextlib.ExitStack() as ctx:
            sb = ctx.enter_context(tc.tile_pool(name='sb', bufs=1))
            ps = ctx.enter_context(tc.tile_pool(name='ps', bufs=2, space='PSUM'))

            _psn = [0]

            def pstile(tag='attq'):
                _psn[0] += 1
                bufs = 1 if tag in ('tp', 'sc') else 2
                return ps.tile([128, 512], F32, tag=tag, bufs=bufs,
                               name=f'ps{_psn[0]}')

            # ---------------- constants ----------------
            ident = sb.tile([128, 128], F32, tag='ident')
            make_identity(nc, ident)
            ident_bf = sb.tile([128, 128], BF16, tag='ident_bf')
            nc.vector.tensor_copy(out=ident_bf, in_=ident)
            onesf = sb.tile([1, 128], F32, tag='onesf')
            nc.vector.memset(onesf, 1.0)
            ones128 = sb.tile([128, 64], F32, tag='ones128')
            nc.vector.memset(ones128, 1.0)
            ones_row = sb.tile([1, 128], EDT, tag='ones_row')
            nc.vector.tensor_copy(out=ones_row, in_=onesf)
            ones_bf = sb.tile([1, 128], TDT, tag='ones_bf')
            nc.vector.tensor_copy(out=ones_bf, in_=onesf)
            eps = sb.tile([128, 1], F32, tag='eps')
            nc.vector.memset(eps, 1e-5)
            # host-built constants (partition-base rules forbid on-device)
            maskc = sb.tile([128, 3, NQ], TDT, tag='maskc')
            nc.sync.dma_start(out=maskc, in_=maskc_d[:, :, :])
            ohp = sb.tile([8, 512], F32R, tag='ohp')
            nc.sync.dma_start(out=ohp, in_=ohp_d[:, :])

            # ---------------- persistent activations ----------------
            tok = sb.tile([128, NTILE, D], F32, tag='tok')       # residual
            ot = sb.tile([128, 4, TT], TDT, tag='ot')            # attn out^T
            nc.vector.memset(ot[:, :, NT:TT], 0.0)
            # V tiles persist; their all-ones 65th columns are set once
            vimga = sb.tile([128, NIMG, 2, H * 65], TDT, tag='vimga')
            vl4 = sb.tile([1, NIMG, H * 65], TDT, tag='vl4')
            nc.vector.tensor_copy(
                out=vimga.rearrange(
                    'p i c (h e) -> p i c h e', e=65)[:, :, :, :, 64:65],
                in_=ones128[:, 0:NIMG * 2 * H])
            nc.vector.tensor_copy(
                out=vl4.rearrange(
                    'p i (h e) -> p i h e', e=65)[0:1, :, :, 64:65],
                in_=ones128[0:1, 0:NIMG * H])

            # int consts for the DVE Newton rsqrt
            magic = sb.tile([128, NTILE], mybir.dt.int32, tag='magic')
            nc.vector.memset(magic, 0x5f3759df)
            one_i = sb.tile([128, NTILE], mybir.dt.int32, tag='one_i')
            nc.vector.memset(one_i, 1)

            # rsqrt(v+eps) on the DVE (bit-trick seed + 2 Newton steps) --
            # keeps ScalarE free of Sqrt/Ln so its activation-table set
            # never thrashes between the attention Exp and the MLP Gelu.
            def _newton_rsqrt(yy, var_ap, nw):
                """yy: [128, 4, nw] scratch; writes rsqrt into yy[:, 3, :]."""
                veps = yy[:, 0, 0:nw]
                nc.vector.tensor_scalar_add(veps, var_ap, 1e-5)
                yi = yy.bitcast(mybir.dt.int32)
                nc.vector.tensor_tensor(out=yi[:, 1, 0:nw],
                                        in0=yi[:, 0, 0:nw],
                                        in1=one_i[:, 0:nw],
                                        op=ALU.logical_shift_right)
                nc.vector.tensor_tensor(out=yi[:, 1, 0:nw],
                                        in0=magic[:, 0:nw],
                                        in1=yi[:, 1, 0:nw], op=ALU.subtract)
                for it in (2, 3):
                    y = yy[:, it - 1, 0:nw]
                    nc.vector.tensor_tensor(out=yy[:, it, 0:nw], in0=y, in1=y,
                                            op=ALU.mult)
                    nc.vector.tensor_tensor(out=yy[:, it, 0:nw],
                                            in0=yy[:, it, 0:nw], in1=veps,
                                            op=ALU.mult)
                    nc.vector.tensor_scalar(out=yy[:, it, 0:nw],
                                            in0=yy[:, it, 0:nw],
                                            scalar1=-0.5, scalar2=1.5,
                                            op0=ALU.mult, op1=ALU.add)
                    nc.vector.tensor_tensor(out=yy[:, it, 0:nw], in0=y,
                                            in1=yy[:, it, 0:nw], op=ALU.mult)

            # per-tile LN helper (embed + head)
            def layer_norm_apply(src_ap, dst_ap, n_rows=128):
                nr = slice(0, n_rows)
                stats = sb.tile([128, 6], F32, tag='lnstat', bufs=4)
                mv = sb.tile([128, 2], F32, tag='lnmv', bufs=4)
                nc.vector.bn_stats(out=stats[nr], in_=src_ap)
                nc.vector.bn_aggr(out=mv[nr], in_=stats[nr])
                yy = sb.tile([128, 4, 1], F32, tag='lnyy', bufs=4)
                _newton_rsqrt(yy, mv[:, 1:2], 1)
                rstd = yy[nr, 3, 0:1]
                nmr = sb.tile([128, 1], F32, tag='lnnmr', bufs=4)
                nc.vector.scalar_tensor_tensor(
                    out=nmr[nr], in0=mv[nr, 0:1], scalar=-1.0,
                    in1=rstd, op0=ALU.mult, op1=ALU.mult)
                nc.scalar.activation(out=dst_ap, in_=src_ap, func=AF.Identity,
                                     scale=rstd, bias=nmr[nr])

            # batched LN for a whole 9-tile transformer phase: stats per
            # tile, then ONE [128, 9] rsqrt chain (the per-tile version puts
            # ~2.5us of serial [128,1] DVE ops on the critical path per tile
            # and starves the PE between phases).
            def ln_phase_stats(t0, t1):
                nw = t1 - t0
                mvall = sb.tile([128, 2, NTILE], F32, tag='mvall', bufs=4)
                for t in range(t0, t1):
                    stats = sb.tile([128, 6], F32, tag='lnstat', bufs=4)
                    nc.vector.bn_stats(out=stats, in_=tok[:, t, :])
                    nc.vector.bn_aggr(out=mvall[:, :, t - t0], in_=stats)
                yy = sb.tile([128, 4, NTILE], F32, tag='lnyyb', bufs=4)
                _newton_rsqrt(yy, mvall[:, 1, 0:nw], nw)
                nmr = sb.tile([128, NTILE], F32, tag='lnnmrb', bufs=4)
                nc.vector.tensor_tensor(out=nmr[:, 0:nw],
                                        in0=mvall[:, 0, 0:nw],
                                        in1=yy[:, 3, 0:nw], op=ALU.mult)
                nc.vector.tensor_scalar_mul(nmr[:, 0:nw], nmr[:, 0:nw], -1.0)
                return yy, nmr, t0

            def ln_phase_apply(bb, t, dst_ap):
                yy, nmr, t0 = bb
                nc.scalar.activation(out=dst_ap, in_=tok[:, t, :],
                                     func=AF.Identity,
                                     scale=yy[:, 3, t - t0:t - t0 + 1],
                                     bias=nmr[:, t - t0:t - t0 + 1])

            # ================= patch embed (f32r, as before) =================
            with tc.tile_pool(name='emb', bufs=1) as emb:
                def psetile(tag='attq'):
                    return pstile(tag)

                xt_sb = []
                for kc, k0, kn in ((0, 0, 128), (1, 128, 128), (2, 256, 64)):
                    t_ = emb.tile([kn, TP], EDT, tag=f'xt{kc}')
                    nc.sync.dma_start(out=t_, in_=xt_d[k0:k0 + kn, :])
                    xt_sb.append(t_)
                wc_sb = []
                for kc, k0, kn in ((0, 0, 128), (1, 128, 128), (2, 256, 64)):
                    t_ = emb.tile([kn, 2 * D], EDT, tag=f'wc{kc}')
                    nc.sync.dma_start(out=t_, in_=wc_d[k0:k0 + kn, :])
                    wc_sb.append(t_)
                bc_sb = emb.tile([1, 2 * D], EDT, tag='bc')
                nc.sync.dma_start(out=bc_sb, in_=bc_d[None, :])
                gb_g = emb.tile([128, 2 * D], F32, tag='gbg')
                nc.sync.dma_start(
                    out=gb_g, in_=gb_d[0][None, :].to_broadcast([128, 2 * D]))
                gb_b = emb.tile([128, 2 * D], F32, tag='gbb')
                nc.sync.dma_start(
                    out=gb_b, in_=gb_d[1][None, :].to_broadcast([128, 2 * D]))
                fw_sb = emb.tile([128, 8, D], EDT, tag='fwsb')
                nc.sync.dma_start(
                    out=fw_sb, in_=fw_d[:, :].rearrange('(c p) n -> p c n', p=128))
                fb_sb = emb.tile([1, D], EDT, tag='fbsb')
                nc.sync.dma_start(out=fb_sb, in_=fb_d[None, :])

                fused_d = nc.dram_tensor('fusedbuf', [TP, D], F32)
                for t in range(TP // 128):      # 8 patch-layout tiles
                    combraw = emb.tile([128, 2 * D], F32, tag='combraw', bufs=2)
                    for nh in range(2):
                        cps = psetile()
                        nc.tensor.matmul(cps, ones_row[0:1, :],
                                         bc_sb[0:1, nh * D:(nh + 1) * D],
                                         start=True, stop=False,
                                         skip_group_check=True)
                        for kc in range(3):
                            nc.tensor.matmul(
                                cps, xt_sb[kc][:, t * 128:(t + 1) * 128],
                                wc_sb[kc][:, nh * D:(nh + 1) * D],
                                start=False, stop=(kc == 2),
                                skip_group_check=True)
                        nc.vector.tensor_copy(
                            out=combraw[:, nh * D:(nh + 1) * D], in_=cps)

                    comb = emb.tile([128, 2 * D], F32, tag='comb', bufs=2)
                    layer_norm_apply(combraw[:, 0:D], comb[:, 0:D])
                    layer_norm_apply(combraw[:, D:2 * D], comb[:, D:2 * D])
                    nc.vector.tensor_mul(comb, comb, gb_g)
                    nc.vector.tensor_add(comb, comb, gb_b)

                    combT = emb.tile([128, 8, 128], EDT, tag='combT', bufs=2)
                    for c in range(8):
                        tp = psetile('tp')
                        nc.tensor.transpose(tp[:, 0:128],
                                            comb[:, c * 128:(c + 1) * 128], ident)
                        nc.vector.tensor_copy(out=combT[:, c, :], in_=tp[:, 0:128])

                    gps = psetile('mlpq')
                    nc.tensor.matmul(gps, ones_row[0:1, :], fb_sb,
                                     start=True, stop=False, skip_group_check=True)
                    for c in range(8):
                        nc.tensor.matmul(gps, combT[:, c, :], fw_sb[:, c, :],
                                         start=False, stop=(c == 7),
                                         skip_group_check=True)
                    gt = emb.tile([128, D], F32, tag='gt', bufs=2)
                    nc.scalar.activation(out=gt, in_=gps, func=AF.Sigmoid)
                    diff = emb.tile([128, D], F32, tag='diff', bufs=2)
                    nc.vector.tensor_sub(diff, comb[:, 0:D], comb[:, D:2 * D])
                    nc.vector.tensor_mul(diff, diff, gt)
                    nc.vector.tensor_add(diff, diff, comb[:, D:2 * D])
                    nc.sync.dma_start(out=fused_d[t * 128:(t + 1) * 128, :],
                                      in_=diff)

                # reshuffle patch-layout fused tokens into transformer layout,
                # zero the cls rows (pos add below then yields cls_eff there)
                nc.vector.memset(tok[:, NTILE - 1, :], 0.0)
                for t in range(NTILE):
                    for (rs, nr, img, pos0) in _tile_segments(t):
                        if pos0 == 0:
                            nc.sync.dma_start(out=tok[rs:rs + 1, t, :],
                                              in_=pos_d[TT - 1:TT, :])
                            rs, nr, pos0 = rs + 1, nr - 1, 1
                        if nr <= 0:
                            continue
                        p0 = img * P + (pos0 - 1)
                        nc.sync.dma_start(out=tok[rs:rs + nr, t, :],
                                          in_=fused_d[p0:p0 + nr, :])
                    postile = emb.tile([128, D], F32, tag='pos', bufs=2)
                    nc.sync.dma_start(out=postile,
                                      in_=pos_d[t * 128:(t + 1) * 128, :])
                    nc.vector.tensor_add(tok[:, t, :], tok[:, t, :], postile)

            # ================= transformer layers (bf16) =================
            lay = ctx.enter_context(tc.tile_pool(name='lay', bufs=1))
            for l in range(NLAYERS):
                qkvw_sb = lay.tile([128, 4, 3 * D], TDT, tag='wt', bufs=3)
                nc.sync.dma_start(
                    out=qkvw_sb,
                    in_=qkvw_d[l].rearrange('(c p) n -> p c n', p=128))
                qkvb_sb = lay.tile([128, 12], F32, tag='qkvb', bufs=2)
                nc.sync.dma_start(
                    out=qkvb_sb,
                    in_=qkvb_d[l].rearrange('(c p) -> p c', p=128))
                qkvbv = lay.tile([1, D], TDT, tag='qkvbv', bufs=2)
                nc.sync.dma_start(out=qkvbv, in_=qkvbv_d[l][None, :])
                temp_sb = lay.tile([128, H], F32, tag='temp', bufs=2)
                nc.sync.dma_start(out=temp_sb,
                                  in_=temp_d[l][None, :].to_broadcast([128, H]))

                # ---- A: LN1 + transpose to feature-major ----
                xT = lay.tile([128, 4, TT], TDT, tag='xT', bufs=2)
                bA1 = ln_phase_stats(0, 3)
                bA2 = ln_phase_stats(3, NTILE)
                for t in range(NTILE):
                    xn = lay.tile([128, D], TDT, tag='xn', bufs=3)
                    ln_phase_apply(bA1 if t < 3 else bA2, t, xn)
                    tp4 = ps.tile([128, 4, 128], TDT, tag='tp', bufs=1,
                                  name=f'tpA{l}_{t}')
                    for c in range(4):
                        nc.tensor.matmul(tp4[:, c, :],
                                         xn[:, c * 128:(c + 1) * 128],
                                         ident_bf, is_transpose=True,
                                         skip_group_check=True)
                    nc.vector.tensor_copy(
                        out=xT[:, :, t * 128:(t + 1) * 128], in_=tp4)

                # ---- B: QKV for all images ----
                qkta = lay.tile([128, NIMG, 8, NQ], TDT, tag='qkta', bufs=1)
                for img in range(NIMG):
                    if img == 1:
                        # batched V for the 4 last tokens (cols 256..1027).
                        # Issued after img0's QKV: its DRAM partition-scatter
                        # roundtrip (engine APs cannot base at partitions
                        # 1-3) has ~8us latency that must hide under the
                        # remaining QKV matmuls, not block the attention
                        # FIFO at PV(img0, h0, chunk2).
                        vp4 = pstile()
                        nc.tensor.matmul(vp4[0:NIMG, :], ones_bf[0:1, 0:NIMG],
                                         qkvbv, start=True, stop=False,
                                         skip_group_check=True)
                        for c in range(4):
                            nc.tensor.matmul(
                                vp4[0:NIMG, :], xT[:, c, P:NT:N],
                                qkvw_sb[:, c, 2 * D:3 * D],
                                start=False, stop=(c == 3),
                                skip_group_check=True)
                        vstage = lay.tile([NIMG, D], TDT, tag='vstage',
                                          bufs=2)
                        nc.vector.tensor_copy(out=vstage, in_=vp4[0:NIMG, :])
                        vld = nc.dram_tensor(f'vl4d{l}', [NIMG, D], TDT)
                        nc.sync.dma_start(out=vld[:, :], in_=vstage)
                        nc.sync.dma_start(
                            out=vl4.rearrange(
                                'p i (h e) -> p i h e', e=65)[0:1, :, :, 0:64],
                            in_=vld.rearrange('i (h e) -> i h e', e=64)[None])
                    io = IMGOFF[img]
                    for fc in range(8):
                        qps = pstile()
                        for c in range(4):
                            nc.tensor.matmul(
                                qps[:, 0:NQ],
                                qkvw_sb[:, c, fc * 128:(fc + 1) * 128],
                                xT[:, c, io:io + NQ],
                                start=(c == 0), stop=(c == 3))
                        nc.scalar.activation(
                            out=qkta[:, img, fc, :], in_=qps[:, 0:NQ],
                            func=AF.Identity, bias=qkvb_sb[:, fc:fc + 1],
                            scale=1.0)
                    for c2 in range(2):
                        vp = pstile()
                        nc.tensor.matmul(vp, ones_bf[0:1, :], qkvbv,
                                         start=True, stop=False,
                                         skip_group_check=True)
                        for c in range(4):
                            nc.tensor.matmul(
                                vp, xT[:, c, io + c2 * 128:io + (c2 + 1) * 128],
                                qkvw_sb[:, c, 2 * D:3 * D],
                                start=False, stop=(c == 3),
                                skip_group_check=True)
                        nc.scalar.copy(
                            out=vimga.rearrange(
                                'p i c (h e) -> p i c h e',
                                e=65)[:, img, c2, :, 0:64],
                            in_=vp)


                # ---- C: attention (transposed softmax) ----
                drb_l = []

                def attn_norm(img):
                    io = IMGOFF[img]
                    rr = lay.tile([8, NQ], F32R, tag='rr', bufs=2)
                    nc.vector.reciprocal(out=rr, in_=drb_l[img])
                    if KDUMP and l == 0 and img == 0:
                        nc.sync.dma_start(out=ddrb_d[:, :], in_=drb_l[img])
                    for fc in range(4):
                        rtp = pstile()
                        nc.tensor.matmul(rtp[:, 0:NQ],
                                         ohp[:, fc * 128:(fc + 1) * 128], rr,
                                         start=True, stop=True,
                                         skip_group_check=True)
                        nc.vector.tensor_mul(ot[:, fc, io:io + NQL],
                                             ot[:, fc, io:io + NQL],
                                             rtp[:, 0:NQL])

                for img in range(NIMG):
                    io = IMGOFF[img]
                    drs = lay.tile([1, 8, NQ], F32, tag='drs', bufs=4)
                    if img >= 2:
                        attn_norm(img - 2)
                    for h in range(H):
                        hr = (h % 2) * 64
                        qfc = h // 2
                        kfc = 4 + h // 2
                        sc = ps.tile([128, 3, 512], F32, tag='sc', bufs=1,
                                     name=f'sc{l}_{img}_{h}')
                        for c in range(3):
                            cm = (128, 128, 1)[c]
                            nc.tensor.matmul(
                                sc[0:cm, c, 0:NQ],
                                qkta[hr:hr + 64, img, kfc,
                                     c * 128:c * 128 + cm],
                                qkta[hr:hr + 64, img, qfc, :],
                                start=True, stop=True, skip_group_check=True)
                        pt = lay.tile([128, 3, NQ], TDT, tag='pt', bufs=3)
                        nc.scalar.activation(out=pt, in_=sc[:, :, 0:NQ],
                                             func=AF.Exp,
                                             scale=temp_sb[:, h:h + 1])
                        meng = nc.gpsimd if (h % 2 == 0) else nc.vector
                        meng.tensor_mul(pt, pt, maskc)
                        if KDUMP and l == 0 and img == 0 and h == 0:
                            nc.sync.dma_start(out=dpt_d[:, :, :], in_=pt)
                        pvp = pstile()
                        for c in range(3):
                            cm = (128, 128, 1)[c]
                            lhs = (vimga[0:128, img, c, h * 65:h * 65 + 65]
                                   if c < 2
                                   else vl4[0:1, img, h * 65:h * 65 + 65])
                            nc.tensor.matmul(
                                pvp[0:65, 0:NQ], lhs, pt[0:cm, c, :],
                                start=(c == 0), stop=(c == 2),
                                skip_group_check=True)
                        nc.vector.tensor_copy(
                            out=ot[hr:hr + 64, h // 2, io:io + NQL],
                            in_=pvp[0:64, 0:NQL])
                        nc.vector.tensor_copy(out=drs[0:1, h, :],
                                              in_=pvp[64:65, 0:NQ])
                    # partition-scatter the 8 denominator rows (via DRAM --
                    # engine APs cannot base at partitions 1-7); the
                    # reciprocal + broadcast happen in a second pass so this
                    # ~8us DMA chain never blocks the PE FIFO ahead of the
                    # next image's score matmuls
                    drd = nc.dram_tensor(f'drd{l}_{img}', [8, NQ], F32)
                    nc.sync.dma_start(out=drd[None, :, :], in_=drs[0:1, :, :])
                    drb = lay.tile([8, NQ], F32, tag='drb', bufs=4)
                    nc.sync.dma_start(out=drb, in_=drd[:, :])
                    drb_l.append(drb)

                attn_norm(NIMG - 2)
                attn_norm(NIMG - 1)

                if KDUMP and l == 0:
                    nc.sync.dma_start(out=dqkta_d[:, :, :, :], in_=qkta)
                    nc.sync.dma_start(out=dot_d[:, :, :], in_=ot)

                # ---- D: proj + residual ----
                projw_sb = lay.tile([128, 4, D], TDT, tag='wt', bufs=3)
                nc.sync.dma_start(
                    out=projw_sb,
                    in_=projw_d[l].rearrange('(c p) n -> p c n', p=128))
                projb_sb = lay.tile([1, D], TDT, tag='projb', bufs=2)
                nc.sync.dma_start(out=projb_sb, in_=projb_d[l][None, :])
                for t in range(NTILE):
                    pp = pstile('mlpq')
                    nc.tensor.matmul(pp, ones_bf[0:1, :], projb_sb,
                                     start=True, stop=False,
                                     skip_group_check=True)
                    for c in range(4):
                        nc.tensor.matmul(pp, ot[:, c, t * 128:(t + 1) * 128],
                                         projw_sb[:, c, :],
                                         start=False, stop=(c == 3),
                                         skip_group_check=True)
                    nc.vector.tensor_add(tok[:, t, :], tok[:, t, :], pp)

                # ---- E: LN2 + transpose ----
                xT = lay.tile([128, 4, TT], TDT, tag='xT', bufs=2)
                bE1 = ln_phase_stats(0, 3)
                bE2 = ln_phase_stats(3, NTILE)
                for t in range(NTILE):
                    xn = lay.tile([128, D], TDT, tag='xn', bufs=3)
                    ln_phase_apply(bE1 if t < 3 else bE2, t, xn)
                    tp4 = ps.tile([128, 4, 128], TDT, tag='tp', bufs=1,
                                  name=f'tpE{l}_{t}')
                    for c in range(4):
                        nc.tensor.matmul(tp4[:, c, :],
                                         xn[:, c * 128:(c + 1) * 128],
                                         ident_bf, is_transpose=True,
                                         skip_group_check=True)
                    nc.vector.tensor_copy(
                        out=xT[:, :, t * 128:(t + 1) * 128], in_=tp4)

                # ---- F/G: MLP ----
                w1_sb = lay.tile([128, 4, MLP], TDT, tag='wt', bufs=3)
                nc.sync.dma_start(
                    out=w1_sb, in_=w1_d[l].rearrange('(c p) n -> p c n', p=128))
                b1_sb = lay.tile([128, 16], F32, tag='b1', bufs=2)
                nc.sync.dma_start(
                    out=b1_sb, in_=b1_d[l].rearrange('(c p) -> p c', p=128))
                w2_sb = lay.tile([128, 16, D], TDT, tag='wt', bufs=3)
                nc.sync.dma_start(
                    out=w2_sb, in_=w2_d[l].rearrange('(c p) n -> p c n', p=128))
                b2_sb = lay.tile([1, D], TDT, tag='b2', bufs=2)
                nc.sync.dma_start(out=b2_sb, in_=b2_d[l][None, :])

                for g in range(3):
                    g0 = g * 384
                    gw = 384
                    hT = lay.tile([128, 16, 384], TDT, tag='hT', bufs=1)
                    for hc in range(16):
                        hp = pstile('mlpq')
                        for c in range(4):
                            nc.tensor.matmul(
                                hp[:, 0:gw],
                                w1_sb[:, c, hc * 128:(hc + 1) * 128],
                                xT[:, c, g0:g0 + gw],
                                start=(c == 0), stop=(c == 3))
                        nc.scalar.activation(
                            out=hT[:, hc, 0:gw], in_=hp[:, 0:gw], func=AF.Gelu,
                            bias=b1_sb[:, hc:hc + 1], scale=1.0)
                    for tr in range(gw // 128):
                        t = (g0 + tr * 128) // 128
                        mp = pstile('mlpq')
                        nc.tensor.matmul(mp, ones_bf[0:1, :], b2_sb,
                                         start=True, stop=False,
                                         skip_group_check=True)
                        for c in range(16):
                            nc.tensor.matmul(
                                mp, hT[:, c, tr * 128:(tr + 1) * 128],
                                w2_sb[:, c, :],
                                start=False, stop=(c == 15),
                                skip_group_check=True)
                        nc.vector.tensor_add(tok[:, t, :], tok[:, t, :], mp)

            if KDUMP:
                for t in range(NTILE):
                    nc.sync.dma_start(out=dtok_d[t, :, :], in_=tok[:, t, :])

            # ================= head =================
            hw_sb = lay.tile([128, 4, 4], EDT, tag='hwsb')
            nc.sync.dma_start(out=hw_sb,
                              in_=hw_d[:, :].rearrange('(c p) n -> p c n', p=128))
            hb_sb = lay.tile([1, 4], EDT, tag='hbsb')
            nc.sync.dma_start(out=hb_sb, in_=hb_d[None, :])

            cls_sb = lay.tile([NIMG, D], F32, tag='cls')
            for img in range(NIMG):
                r = IMGOFF[img]
                nc.sync.dma_start(out=cls_sb[img:img + 1, :],
                                  in_=tok[r % 128:r % 128 + 1, r // 128, :])
            clsn = lay.tile([NIMG, D], F32, tag='clsn')
            layer_norm_apply(cls_sb[0:NIMG, :], clsn[0:NIMG, :], n_rows=NIMG)
            clsT = lay.tile([128, 4, NIMG], EDT, tag='clsT')
            for c in range(4):
                tp = pstile('mlpq')
                nc.tensor.transpose(tp[0:128, 0:NIMG],
                                    clsn[0:NIMG, c * 128:(c + 1) * 128],
                                    ident[0:NIMG, 0:NIMG])
                nc.vector.tensor_copy(out=clsT[:, c, :], in_=tp[0:128, 0:NIMG])
            op = pstile('mlpq')
            nc.tensor.matmul(op[0:NIMG, 0:4], ones_row[0:1, 0:NIMG], hb_sb,
                             start=True, stop=False, skip_group_check=True)
            for c in range(4):
                nc.tensor.matmul(op[0:NIMG, 0:4], clsT[:, c, :],
                                 hw_sb[:, c, :],
                                 start=False, stop=(c == 3),
                                 skip_group_check=True)
            osb = lay.tile([NIMG, 4], F32, tag='osb')
            nc.vector.tensor_copy(out=osb[0:NIMG, :], in_=op[0:NIMG, 0:4])
            nc.sync.dma_start(out=out_d[:, :], in_=osb[0:NIMG, :])

    return nc


# ============================================================================
# entry point
# ============================================================================
def kernel(**inputs) -> np.ndarray:
    _install_fixups()
    from concourse.bass_utils import run_bass_kernel_spmd

    key = ('nc', os.environ.get('KLAYERS', ''), os.environ.get('KDUMP', ''))
    if key not in _PROGRAM_CACHE:
        _PROGRAM_CACHE[key] = _build_program()
    nc = _PROGRAM_CACHE[key]
    _PROGRAM_CACHE['nc'] = nc

    in_maps = _host_prep(inputs)
    res = run_bass_kernel_spmd(nc, in_maps, core_ids=list(range(NCORES)))
    out = np.concatenate([np.asarray(res.results[i]['out'])
                          for i in range(NCORES)], 0)
    return out[:, :NCLS].astype(np.float32)
